# revision 9
# baseline (speedup 1.0000x reference)
"""Trainium2 Bass kernel for the Human3.6M pose postprocess
(spherical->xyz conversion + kinematic-tree accumulation).

Self-contained: hardcodes the problem shapes
  observed_pose (4096, 16, 96) f32, pred_pose (4096, 64, 66) f32
and shards the batch dim across 8 NeuronCores (pure data parallel).

Per-core design (bt-major layout, no transposes, no TensorE):
  - partition p <-> one batch example; free dim = (64 time steps) x channels
  - range reduction to [-pi, pi] without a mod op:
      n  = round(x / 2pi)   via two ACT affine passes (big-constant
                            round-to-nearest trick: +C then -C, C = 1.5*2^23)
      w  = x - 2pi*n        one DVE scalar_tensor_tensor
    theta and phi are processed interleaved (one strided op covers both).
  - sin = Sin(w); cos = Sin(pi/2 - |w|)  (both args within the +-4 LUT range)
  - DVE: muls for spherical->xyz, then ONE gated tensor_tensor_scan per xyz
    component computes the entire 22-edge kinematic tree walk in natural
    output-channel order:  state = gate*state + w;  gate=0 at root channels
    {0,1,6,11} reseeds state from the last observed frame, zero-offset slots
    reproduce the IGNORE copies, and two correction slots (ch16, ch24)
    rewind the state to x[13] across tree branches.
Output leaves the device component-major [nb, 3, 64, 32]; the host
transposes back to (B, T, 96).
"""

import math
import sys

for _p in ("/opt/trn_rl_repo", "/root/.axon_site/_ro/trn_rl_repo"):
    if _p not in sys.path:
        sys.path.insert(0, _p)

import numpy as np

PI = math.pi
BIGC = 1.5 * 2**23  # fp32 round-to-nearest-integer constant
T = 64   # time steps = reps per partition
P = 128  # partitions per tile

N_CORES = 8
B = 4096
NB = B // N_CORES  # batches per core

# child-joint order of CONNECT (k index) -> contiguous runs in output-channel
# space: (k_start, ch_start, length)
ASSEM_RUNS = [
    (0, 12, 4),   # k0..3   -> ch12..15  (spine 12,13,14,15)
    (4, 25, 3),   # k4..6   -> ch25..27  (arm 25,26,27)
    (7, 29, 2),   # k7..8   -> ch29..30  (arm 29,30)
    (9, 17, 3),   # k9..11  -> ch17..19  (arm 17,18,19)
    (12, 21, 2),  # k12..13 -> ch21..22  (arm 21,22)
    (14, 2, 4),   # k14..17 -> ch2..5    (leg 2,3,4,5)
    (18, 7, 4),   # k18..21 -> ch7..10   (leg 7,8,9,10)
]


def build_kernel(nc, n_b: int):
    """Build the postprocess kernel for n_b batch examples on one core."""
    import concourse.tile as tile
    from concourse import mybir

    f32 = mybir.dt.float32
    pred = nc.dram_tensor("pred", [n_b * T, 66], f32, kind="ExternalInput")
    obs = nc.dram_tensor("obs", [n_b, 96], f32, kind="ExternalInput")
    out = nc.dram_tensor("out", [n_b, 3 * T * 32], f32, kind="ExternalOutput")

    with tile.TileContext(nc) as tc:
        build_tile_kernel(tc, pred, obs, out, n_b)
    return nc


def build_tile_kernel(tc, pred, obs, out, n_b: int):
    import concourse.bass as bass
    from concourse import mybir

    f32 = mybir.dt.float32
    ALU = mybir.AluOpType
    ACTF = mybir.ActivationFunctionType
    nc = tc.nc
    nt = (n_b + P - 1) // P
    pp = min(P, n_b)

    # pred rows grouped per tile: partition p holds T consecutive rows (one b)
    pred_t = pred.ap().rearrange("(n p r) c -> n p (r c)", p=pp, r=T)

    with (
        tc.tile_pool(name="io", bufs=2) as io_pool,
        tc.tile_pool(name="mid", bufs=2) as mid_pool,
        tc.tile_pool(name="mid1", bufs=1) as mid1_pool,
        tc.tile_pool(name="const", bufs=1) as const_pool,
    ):
        # static gate tile [pp, 3*T*32] (all three components in one scan):
        # 1.0 everywhere, 0.0 at root channels
        gate = const_pool.tile([pp, 3 * T * 32], f32)
        g4 = gate.rearrange("p (c r j) -> p c r j", c=3, j=32)
        nc.vector.memset(gate, 1.0)
        nc.vector.memset(g4[:, :, :, 0:2], 0.0)       # ch 0, 1
        nc.vector.memset(g4[:, :, :, 6:12:5], 0.0)    # ch 6, 11

        # per-partition constant for Sin biases
        halfpi = const_pool.tile([pp, 1], f32)
        nc.vector.memset(halfpi, PI / 2)

        for i in range(nt):
            b0 = i * pp

            raw = io_pool.tile([pp, T * 66], f32)
            nc.sync.dma_start(out=raw, in_=pred_t[i])
            r4 = raw.rearrange("p (r c) -> p r c", c=66)
            # interleaved (theta, phi) strided view: offsets 1,2 of each joint
            th_ph = bass.AP(
                tensor=raw.tensor,
                offset=raw.offset + 1,
                ap=[raw.ap[0], [66, T], [3, 22], [1, 2]],
            )

            # ---- range reduction: n = round(x/2pi), w = x - 2pi*n ----
            # (in-place chain: ang ends up holding w)
            ang = mid_pool.tile([pp, T, 22, 2], f32, tag="ang")
            nc.scalar.activation(out=ang[:, :, :, :], in_=th_ph, func=ACTF.Copy,
                                 bias=BIGC, scale=1.0 / (2 * PI))
            nc.scalar.activation(out=ang[:, :, :, :], in_=ang[:, :, :, :],
                                 func=ACTF.Copy, bias=-BIGC, scale=1.0)
            nc.vector.scalar_tensor_tensor(
                out=ang[:, :, :, :], in0=ang[:, :, :, :], scalar=-2 * PI,
                in1=th_ph, op0=ALU.mult, op1=ALU.add,
            )
            # ---- trig: sin = Sin(w); cos = Sin(pi/2 - |w|) ----
            # TG blocks: [cos_t, sin_t, sin_p, cos_p], each [T*22]
            absw = mid1_pool.tile([pp, T, 22, 2], f32)
            nc.scalar.activation(out=absw[:, :, :, :], in_=ang[:, :, :, :],
                                 func=ACTF.Abs)
            tg = mid1_pool.tile([pp, 4, T * 22], f32)
            KT = T * 22
            nc.scalar.activation(
                out=bass.AP(tensor=tg.tensor, offset=tg.offset + KT,
                            ap=[tg.ap[0], [22, T], [1, 22], [KT, 2]]),
                in_=ang[:, :, :, :], func=ACTF.Sin, bias=0.0, scale=1.0)
            nc.scalar.activation(
                out=bass.AP(tensor=tg.tensor, offset=tg.offset,
                            ap=[tg.ap[0], [22, T], [1, 22], [3 * KT, 2]]),
                in_=absw[:, :, :, :], func=ACTF.Sin,
                bias=halfpi[:, 0:1], scale=-1.0)

            # ---- spherical -> xyz, paired muls on dense operands ----
            # rd = dense copy of r (GPSIMD); Q blocks: [rsin, x0, x1, x2]
            rd = mid1_pool.tile([pp, T, 22], f32)
            nc.gpsimd.tensor_copy(out=rd, in_=r4[:, :, 0:66:3])
            q = mid1_pool.tile([pp, 4, T, 22], f32)
            rd2 = bass.AP(tensor=rd.tensor, offset=rd.offset,
                          ap=[rd.ap[0], [0, 2], [1, KT]])
            # [rsin, x1] = [r, r] * [sin_p, cos_p]
            nc.vector.tensor_tensor(
                out=bass.AP(tensor=q.tensor, offset=q.offset,
                            ap=[q.ap[0], [2 * KT, 2], [1, KT]]),
                in0=rd2, in1=tg[:, 2:4], op=ALU.mult)
            # [x0, x2] = [rsin, rsin] * [cos_t, sin_t]
            nc.vector.tensor_tensor(
                out=bass.AP(tensor=q.tensor, offset=q.offset + KT,
                            ap=[q.ap[0], [2 * KT, 2], [1, KT]]),
                in0=bass.AP(tensor=q.tensor, offset=q.offset,
                            ap=[q.ap[0], [0, 2], [1, KT]]),
                in1=tg[:, 0:2], op=ALU.mult)
            xyz = q[:, 1:4]  # (comp, rep, k) with comps x0, x1, x2

            # ---- assemble scan work buffer W [pp, 3, T, 32] ----
            w = io_pool.tile([pp, 3, T, 32], f32)
            # zero-offset slots (ch 20,28 and 23,31 = IGNORE copies)
            nc.gpsimd.memset(w[:, :, :, 20:29:8], 0.0)
            nc.gpsimd.memset(w[:, :, :, 23:32:8], 0.0)
            # root slots from obs: ch{0,1} <- cols 0..5, ch{6,11} <- 18..20/33..35
            obs_t = mid_pool.tile([pp, 96], f32)
            nc.sync.dma_start(out=obs_t, in_=obs[b0 : b0 + pp, :])
            nc.gpsimd.tensor_copy(
                out=w[:, :, :, 0:2],
                in_=bass.AP(tensor=obs_t.tensor, offset=obs_t.offset,
                            ap=[obs_t.ap[0], [1, 3], [0, T], [3, 2]]),
            )
            nc.gpsimd.tensor_copy(
                out=w[:, :, :, 6:12:5],
                in_=bass.AP(tensor=obs_t.tensor, offset=obs_t.offset + 18,
                            ap=[obs_t.ap[0], [1, 3], [0, T], [15, 2]]),
            )
            # xyz offset slots (7 contiguous runs) via SBUF->SBUF DMA
            for k0, ch0, ln in ASSEM_RUNS:
                nc.sync.dma_start(
                    out=w[:, :, :, ch0 : ch0 + ln], in_=xyz[:, :, :, k0 : k0 + ln]
                )
            # correction slots: ch16 = -(k2+k3) -> x13; ch24 = -(k9..k13) -> x13
            nc.vector.tensor_reduce(
                out=w[:, :, :, 16:17], in_=xyz[:, :, :, 2:4],
                axis=mybir.AxisListType.X, op=ALU.add, negate=True)
            nc.vector.tensor_reduce(
                out=w[:, :, :, 24:25], in_=xyz[:, :, :, 9:14],
                axis=mybir.AxisListType.X, op=ALU.add, negate=True)

            # ---- gated scan, all 3 components fused (in place) ----
            w2 = w.rearrange("p c r j -> p (c r j)")
            nc.vector.tensor_tensor_scan(
                out=w2, data0=gate, data1=w2,
                initial=0.0, op0=ALU.mult, op1=ALU.add)

            nc.sync.dma_start(
                out=out[b0 : b0 + pp, :],
                in_=w.rearrange("p c r j -> p (c r j)"),
            )


_CACHE = {}


def _get_nc():
    if "nc" not in _CACHE:
        import concourse.bacc as bacc

        nc = bacc.Bacc("TRN2", target_bir_lowering=False)
        build_kernel(nc, NB)
        nc.compile()
        _CACHE["nc"] = nc
    return _CACHE["nc"]


def _run(in_maps, **kwargs):
    from concourse.bass_utils import run_bass_kernel_spmd

    nc = _get_nc()
    return run_bass_kernel_spmd(nc, in_maps, core_ids=list(range(N_CORES)), **kwargs)


def _make_in_maps(observed_pose, pred_pose):
    obs_last = np.ascontiguousarray(observed_pose[:, -1, :], dtype=np.float32)
    pred = np.ascontiguousarray(pred_pose, dtype=np.float32)
    in_maps = []
    for c in range(N_CORES):
        in_maps.append(
            {
                "pred": np.ascontiguousarray(
                    pred[c * NB : (c + 1) * NB].reshape(NB * T, 66)
                ),
                "obs": obs_last[c * NB : (c + 1) * NB],
            }
        )
    return in_maps


def _assemble_out(results):
    outs = []
    for c in range(N_CORES):
        o = results[c]["out"].reshape(NB, 3, T, 32)
        outs.append(o.transpose(0, 2, 3, 1).reshape(NB, T, 96))
    return np.ascontiguousarray(np.concatenate(outs, axis=0), dtype=np.float32)


def kernel(observed_pose, pred_pose):
    res = _run(_make_in_maps(observed_pose, pred_pose))
    return _assemble_out(res.results)


def kernel_traced(observed_pose, pred_pose, trace_cores=None):
    """Like kernel() but returns (output, BassKernelResults) with an NTFF trace."""
    res = _run(
        _make_in_maps(observed_pose, pred_pose),
        trace=True,
        trace_cores=trace_cores or [0],
    )
    return _assemble_out(res.results), res


# revision 12
# speedup vs baseline: 3.2960x; 3.2960x over previous
"""Trainium2 Bass kernel for the Human3.6M pose postprocess
(spherical->xyz conversion + kinematic-tree accumulation).

Self-contained: hardcodes the problem shapes
  observed_pose (4096, 16, 96) f32, pred_pose (4096, 64, 66) f32
and shards the batch dim across 8 NeuronCores (pure data parallel).

Per-core design (bt-major layout, no transposes, no TensorE):
  - partition p <-> one batch example; free dim = (64 time steps) x channels
  - range reduction to [-pi, pi] without a mod op:
      n  = round(x / 2pi)   via two ACT affine passes (big-constant
                            round-to-nearest trick: +C then -C, C = 1.5*2^23)
      w  = x - 2pi*n        one DVE scalar_tensor_tensor
    theta and phi are processed interleaved (one strided op covers both).
  - sin = Sin(w); cos = Sin(pi/2 - |w|)  (both args within the +-4 LUT range)
  - DVE: muls for spherical->xyz, then ONE gated tensor_tensor_scan per xyz
    component computes the entire 22-edge kinematic tree walk in natural
    output-channel order:  state = gate*state + w;  gate=0 at root channels
    {0,1,6,11} reseeds state from the last observed frame, zero-offset slots
    reproduce the IGNORE copies, and two correction slots (ch16, ch24)
    rewind the state to x[13] across tree branches.
Output leaves the device component-major [nb, 3, 64, 32]; the host
transposes back to (B, T, 96).
"""

import math
import sys

for _p in ("/opt/trn_rl_repo", "/root/.axon_site/_ro/trn_rl_repo"):
    if _p not in sys.path:
        sys.path.insert(0, _p)

import numpy as np

PI = math.pi
BIGC = 1.5 * 2**23  # fp32 round-to-nearest-integer constant
T = 64   # time steps = reps per partition
P = 128  # partitions per tile

N_CORES = 8
B = 4096
NB = B // N_CORES  # batches per core

# child-joint order of CONNECT (k index) -> contiguous runs in output-channel
# space: (k_start, ch_start, length)
ASSEM_RUNS = [
    (0, 12, 4),   # k0..3   -> ch12..15  (spine 12,13,14,15)
    (4, 25, 3),   # k4..6   -> ch25..27  (arm 25,26,27)
    (7, 29, 2),   # k7..8   -> ch29..30  (arm 29,30)
    (9, 17, 3),   # k9..11  -> ch17..19  (arm 17,18,19)
    (12, 21, 2),  # k12..13 -> ch21..22  (arm 21,22)
    (14, 2, 4),   # k14..17 -> ch2..5    (leg 2,3,4,5)
    (18, 7, 4),   # k18..21 -> ch7..10   (leg 7,8,9,10)
]


def build_kernel(nc, n_b: int):
    """Build the postprocess kernel for n_b batch examples on one core."""
    import concourse.tile as tile
    from concourse import mybir

    f32 = mybir.dt.float32
    pred = nc.dram_tensor("pred", [n_b * T, 66], f32, kind="ExternalInput")
    obs = nc.dram_tensor("obs", [n_b, 96], f32, kind="ExternalInput")
    out = nc.dram_tensor("out", [n_b, 3 * T * 32], f32, kind="ExternalOutput")

    with tile.TileContext(nc) as tc:
        build_tile_kernel(tc, pred, obs, out, n_b)
    return nc


def build_tile_kernel(tc, pred, obs, out, n_b: int):
    import concourse.bass as bass
    from concourse import mybir

    f32 = mybir.dt.float32
    ALU = mybir.AluOpType
    ACTF = mybir.ActivationFunctionType
    nc = tc.nc
    nt = (n_b + P - 1) // P
    pp = min(P, n_b)

    # pred rows grouped per tile: partition p holds T consecutive rows (one b)
    pred_t = pred.ap().rearrange("(n p r) c -> n p (r c)", p=pp, r=T)

    with (
        tc.tile_pool(name="io", bufs=2) as io_pool,
        tc.tile_pool(name="mid", bufs=2) as mid_pool,
        tc.tile_pool(name="mid1", bufs=1) as mid1_pool,
        tc.tile_pool(name="const", bufs=1) as const_pool,
    ):
        # static gate tile [pp, 3*T*32] (all three components in one scan):
        # 1.0 everywhere, 0.0 at root channels
        gate = const_pool.tile([pp, 3 * T * 32], f32)
        g4 = gate.rearrange("p (c r j) -> p c r j", c=3, j=32)
        nc.vector.memset(gate, 1.0)
        nc.vector.memset(g4[:, :, :, 0:2], 0.0)       # ch 0, 1
        nc.vector.memset(g4[:, :, :, 6:12:5], 0.0)    # ch 6, 11

        # per-partition constant for Sin biases
        halfpi = const_pool.tile([pp, 1], f32)
        nc.vector.memset(halfpi, PI / 2)

        for i in range(nt):
            b0 = i * pp

            raw = io_pool.tile([pp, T * 66], f32)
            nc.sync.dma_start(out=raw, in_=pred_t[i])
            r4 = raw.rearrange("p (r c) -> p r c", c=66)
            # interleaved (theta, phi) strided view: offsets 1,2 of each joint
            th_ph = bass.AP(
                tensor=raw.tensor,
                offset=raw.offset + 1,
                ap=[raw.ap[0], [66, T], [3, 22], [1, 2]],
            )

            # ---- trig via half-angle, no range reduction needed ----
            # |x/2| <= 2.82 < 4 (Sin LUT limit).  With s = Sin(x/2),
            # c = Sin(x/2 + pi/2):  sin x = 2sc,  cos x = 1 - 2s^2.
            # The 2x/4x factors fold into rd4 = 4r and the affine biases.
            KT = T * 22
            # TGH blocks: [s_t | s_p | c_t | c_p] (split halves, dense)
            tgh = mid1_pool.tile([pp, 4, KT], f32)
            half_out0 = bass.AP(tensor=tgh.tensor, offset=tgh.offset,
                                ap=[tgh.ap[0], [22, T], [1, 22], [KT, 2]])
            half_out2 = bass.AP(tensor=tgh.tensor, offset=tgh.offset + 2 * KT,
                                ap=[tgh.ap[0], [22, T], [1, 22], [KT, 2]])
            nc.scalar.activation(out=half_out0, in_=th_ph, func=ACTF.Sin,
                                 bias=0.0, scale=0.5)
            # cos(x/2) = Sin(pi/2 - |x|/2): the |.| keeps the argument in
            # the LUT's +-4 range even for |x| up to 5.7
            absx = mid1_pool.tile([pp, T, 22, 2], f32)
            nc.scalar.activation(out=absx[:, :, :, :], in_=th_ph, func=ACTF.Abs)
            nc.scalar.activation(out=half_out2, in_=absx[:, :, :, :],
                                 func=ACTF.Sin, bias=halfpi[:, 0:1], scale=-0.5)

            # TG2 blocks: [ct~ | st~ | sp~ | cp~] where
            #   st~ = s_t*c_t (sin t = 2 st~), ct~ = 0.5 - s_t^2 (cos t = 2 ct~)
            #   sp~ = s_p*c_p,                 cp~ = 0.25 - 0.5 s_p^2
            tg2 = mid1_pool.tile([pp, 4, KT], f32)
            nc.vector.tensor_tensor(
                out=bass.AP(tensor=tg2.tensor, offset=tg2.offset + KT,
                            ap=[tg2.ap[0], [KT, 2], [1, KT]]),
                in0=tgh[:, 0:2], in1=tgh[:, 2:4], op=ALU.mult)
            # squares in place over TGH[0:2] (after the product above)
            nc.scalar.activation(out=tgh[:, 0:2], in_=tgh[:, 0:2],
                                 func=ACTF.Square)
            nc.scalar.activation(out=tg2[:, 0], in_=tgh[:, 0],
                                 func=ACTF.Copy, bias=0.5, scale=-1.0)
            nc.scalar.activation(out=tg2[:, 3], in_=tgh[:, 1],
                                 func=ACTF.Copy, bias=0.25, scale=-0.5)

            # rd4 = 4r dense (ACT)
            rd = mid1_pool.tile([pp, T, 22], f32)
            nc.scalar.activation(out=rd, in_=r4[:, :, 0:66:3],
                                 func=ACTF.Copy, bias=0.0, scale=4.0)
            # Q blocks: [rsin4, x0, x1, x2]; rsin4 = 4r*sp~ = 2 r sin(phi)
            q = mid1_pool.tile([pp, 4, T, 22], f32)
            rd2 = bass.AP(tensor=rd.tensor, offset=rd.offset,
                          ap=[rd.ap[0], [0, 2], [1, KT]])
            # [rsin4, x1] = [4r, 4r] * [sp~, cp~]   (GPSIMD)
            nc.gpsimd.tensor_tensor(
                out=bass.AP(tensor=q.tensor, offset=q.offset,
                            ap=[q.ap[0], [2 * KT, 2], [1, KT]]),
                in0=rd2, in1=tg2[:, 2:4], op=ALU.mult)
            # [x0, x2] = [rsin4, rsin4] * [ct~, st~]   (DVE)
            nc.vector.tensor_tensor(
                out=bass.AP(tensor=q.tensor, offset=q.offset + KT,
                            ap=[q.ap[0], [2 * KT, 2], [1, KT]]),
                in0=bass.AP(tensor=q.tensor, offset=q.offset,
                            ap=[q.ap[0], [0, 2], [1, KT]]),
                in1=tg2[:, 0:2], op=ALU.mult)
            xyz = q[:, 1:4]  # (comp, rep, k) with comps x0, x1, x2

            # ---- assemble scan work buffer W [pp, 3, T, 32] ----
            w = io_pool.tile([pp, 3, T, 32], f32)
            # zero-offset slots (ch 20,28 and 23,31 = IGNORE copies)
            nc.gpsimd.memset(w[:, :, :, 20:29:8], 0.0)
            nc.gpsimd.memset(w[:, :, :, 23:32:8], 0.0)
            # root slots from obs: ch{0,1} <- cols 0..5, ch{6,11} <- 18..20/33..35
            obs_t = mid_pool.tile([pp, 96], f32)
            nc.sync.dma_start(out=obs_t, in_=obs[b0 : b0 + pp, :])
            nc.gpsimd.tensor_copy(
                out=w[:, :, :, 0:2],
                in_=bass.AP(tensor=obs_t.tensor, offset=obs_t.offset,
                            ap=[obs_t.ap[0], [1, 3], [0, T], [3, 2]]),
            )
            nc.gpsimd.tensor_copy(
                out=w[:, :, :, 6:12:5],
                in_=bass.AP(tensor=obs_t.tensor, offset=obs_t.offset + 18,
                            ap=[obs_t.ap[0], [1, 3], [0, T], [15, 2]]),
            )
            # xyz offset slots (7 contiguous runs) on ACT
            for k0, ch0, ln in ASSEM_RUNS:
                nc.scalar.copy(
                    out=w[:, :, :, ch0 : ch0 + ln], in_=xyz[:, :, :, k0 : k0 + ln]
                )
            # correction slots: ch16 = -(k2+k3) -> x13; ch24 = -(k9..k13) -> x13
            nc.vector.tensor_reduce(
                out=w[:, :, :, 16:17], in_=xyz[:, :, :, 2:4],
                axis=mybir.AxisListType.X, op=ALU.add, negate=True)
            nc.vector.tensor_reduce(
                out=w[:, :, :, 24:25], in_=xyz[:, :, :, 9:14],
                axis=mybir.AxisListType.X, op=ALU.add, negate=True)

            # ---- gated scan, all 3 components fused (in place) ----
            w2 = w.rearrange("p c r j -> p (c r j)")
            nc.vector.tensor_tensor_scan(
                out=w2, data0=gate, data1=w2,
                initial=0.0, op0=ALU.mult, op1=ALU.add)

            nc.sync.dma_start(
                out=out[b0 : b0 + pp, :],
                in_=w.rearrange("p c r j -> p (c r j)"),
            )


_CACHE = {}


def _get_nc():
    if "nc" not in _CACHE:
        import concourse.bacc as bacc

        nc = bacc.Bacc("TRN2", target_bir_lowering=False)
        build_kernel(nc, NB)
        nc.compile()
        _CACHE["nc"] = nc
    return _CACHE["nc"]


def _run(in_maps, **kwargs):
    from concourse.bass_utils import run_bass_kernel_spmd

    nc = _get_nc()
    return run_bass_kernel_spmd(nc, in_maps, core_ids=list(range(N_CORES)), **kwargs)


def _make_in_maps(observed_pose, pred_pose):
    obs_last = np.ascontiguousarray(observed_pose[:, -1, :], dtype=np.float32)
    pred = np.ascontiguousarray(pred_pose, dtype=np.float32)
    in_maps = []
    for c in range(N_CORES):
        in_maps.append(
            {
                "pred": np.ascontiguousarray(
                    pred[c * NB : (c + 1) * NB].reshape(NB * T, 66)
                ),
                "obs": obs_last[c * NB : (c + 1) * NB],
            }
        )
    return in_maps


def _assemble_out(results):
    outs = []
    for c in range(N_CORES):
        o = results[c]["out"].reshape(NB, 3, T, 32)
        outs.append(o.transpose(0, 2, 3, 1).reshape(NB, T, 96))
    return np.ascontiguousarray(np.concatenate(outs, axis=0), dtype=np.float32)


def kernel(observed_pose, pred_pose):
    res = _run(_make_in_maps(observed_pose, pred_pose))
    return _assemble_out(res.results)


def kernel_traced(observed_pose, pred_pose, trace_cores=None):
    """Like kernel() but returns (output, BassKernelResults) with an NTFF trace."""
    res = _run(
        _make_in_maps(observed_pose, pred_pose),
        trace=True,
        trace_cores=trace_cores or [0],
    )
    return _assemble_out(res.results), res


# revision 15
# speedup vs baseline: 3.3092x; 1.0040x over previous
"""Trainium2 Bass kernel for the Human3.6M pose postprocess
(spherical->xyz conversion + kinematic-tree accumulation).

Self-contained: hardcodes the problem shapes
  observed_pose (4096, 16, 96) f32, pred_pose (4096, 64, 66) f32
and shards the batch dim across 8 NeuronCores (pure data parallel).

Per-core design (bt-major layout, no transposes, no TensorE):
  - partition p <-> one batch example; free dim = (64 time steps) x channels
  - range reduction to [-pi, pi] without a mod op:
      n  = round(x / 2pi)   via two ACT affine passes (big-constant
                            round-to-nearest trick: +C then -C, C = 1.5*2^23)
      w  = x - 2pi*n        one DVE scalar_tensor_tensor
    theta and phi are processed interleaved (one strided op covers both).
  - sin = Sin(w); cos = Sin(pi/2 - |w|)  (both args within the +-4 LUT range)
  - DVE: muls for spherical->xyz, then ONE gated tensor_tensor_scan per xyz
    component computes the entire 22-edge kinematic tree walk in natural
    output-channel order:  state = gate*state + w;  gate=0 at root channels
    {0,1,6,11} reseeds state from the last observed frame, zero-offset slots
    reproduce the IGNORE copies, and two correction slots (ch16, ch24)
    rewind the state to x[13] across tree branches.
Output leaves the device component-major [nb, 3, 64, 32]; the host
transposes back to (B, T, 96).
"""

import math
import sys

for _p in ("/opt/trn_rl_repo", "/root/.axon_site/_ro/trn_rl_repo"):
    if _p not in sys.path:
        sys.path.insert(0, _p)

import numpy as np

PI = math.pi
BIGC = 1.5 * 2**23  # fp32 round-to-nearest-integer constant
T = 64   # time steps = reps per partition
P = 128  # partitions per tile

N_CORES = 8
B = 4096
NB = B // N_CORES  # batches per core

# child-joint order of CONNECT (k index) -> contiguous runs in output-channel
# space: (k_start, ch_start, length)
ASSEM_RUNS = [
    (0, 12, 4),   # k0..3   -> ch12..15  (spine 12,13,14,15)
    (4, 25, 3),   # k4..6   -> ch25..27  (arm 25,26,27)
    (7, 29, 2),   # k7..8   -> ch29..30  (arm 29,30)
    (9, 17, 3),   # k9..11  -> ch17..19  (arm 17,18,19)
    (12, 21, 2),  # k12..13 -> ch21..22  (arm 21,22)
    (14, 2, 4),   # k14..17 -> ch2..5    (leg 2,3,4,5)
    (18, 7, 4),   # k18..21 -> ch7..10   (leg 7,8,9,10)
]


def build_kernel(nc, n_b: int):
    """Build the postprocess kernel for n_b batch examples on one core."""
    import concourse.tile as tile
    from concourse import mybir

    f32 = mybir.dt.float32
    pred = nc.dram_tensor("pred", [n_b * T, 66], f32, kind="ExternalInput")
    obs = nc.dram_tensor("obs", [n_b, 96], f32, kind="ExternalInput")
    out = nc.dram_tensor("out", [n_b, 3 * T * 32], f32, kind="ExternalOutput")

    with tile.TileContext(nc) as tc:
        build_tile_kernel(tc, pred, obs, out, n_b)
    return nc


def build_tile_kernel(tc, pred, obs, out, n_b: int):
    import concourse.bass as bass
    from concourse import mybir

    f32 = mybir.dt.float32
    ALU = mybir.AluOpType
    ACTF = mybir.ActivationFunctionType
    nc = tc.nc
    nt = (n_b + P - 1) // P
    pp = min(P, n_b)

    # pred rows grouped per tile: partition p holds T consecutive rows (one b)
    pred_t = pred.ap().rearrange("(n p r) c -> n p (r c)", p=pp, r=T)

    with (
        tc.tile_pool(name="io", bufs=2) as io_pool,
        tc.tile_pool(name="mid", bufs=2) as mid_pool,
        tc.tile_pool(name="mid1", bufs=1) as mid1_pool,
        tc.tile_pool(name="const", bufs=1) as const_pool,
    ):
        # static gate tile [pp, 3*T*32] (all three components in one scan):
        # 1.0 everywhere, 0.0 at root channels
        gate = const_pool.tile([pp, 3 * T * 32], f32)
        g4 = gate.rearrange("p (c r j) -> p c r j", c=3, j=32)
        nc.vector.memset(gate, 1.0)
        nc.vector.memset(g4[:, :, :, 0:2], 0.0)       # ch 0, 1
        nc.vector.memset(g4[:, :, :, 6:12:5], 0.0)    # ch 6, 11

        # per-partition constant for Sin biases
        halfpi = const_pool.tile([pp, 1], f32)
        nc.vector.memset(halfpi, PI / 2)

        for i in range(nt):
            b0 = i * pp

            raw = io_pool.tile([pp, T * 66], f32)
            nc.sync.dma_start(out=raw, in_=pred_t[i])
            r4 = raw.rearrange("p (r c) -> p r c", c=66)
            # interleaved (theta, phi) strided view: offsets 1,2 of each joint
            th_ph = bass.AP(
                tensor=raw.tensor,
                offset=raw.offset + 1,
                ap=[raw.ap[0], [66, T], [3, 22], [1, 2]],
            )

            # ---- trig via half-angle, no range reduction needed ----
            # |x/2| <= 2.82 < 4 (Sin LUT limit).  With s = Sin(x/2),
            # c = Sin(x/2 + pi/2):  sin x = 2sc,  cos x = 1 - 2s^2.
            # The 2x/4x factors fold into rd4 = 4r and the affine biases.
            KT = T * 22
            # TGH blocks: [s_t | s_p | c_t | c_p] (split halves, dense).
            # Iteration order (pair, rep, k) keeps the OUT inner dim
            # contiguous (stride-1 runs of 1408) — ACT runs at line rate.
            tgh = mid1_pool.tile([pp, 4, KT], f32)
            half_out0 = bass.AP(tensor=tgh.tensor, offset=tgh.offset,
                                ap=[tgh.ap[0], [KT, 2], [22, T], [1, 22]])
            half_out2 = bass.AP(tensor=tgh.tensor, offset=tgh.offset + 2 * KT,
                                ap=[tgh.ap[0], [KT, 2], [22, T], [1, 22]])
            th_ph_pm = bass.AP(tensor=raw.tensor, offset=raw.offset + 1,
                               ap=[raw.ap[0], [1, 2], [66, T], [3, 22]])
            nc.scalar.activation(out=half_out0, in_=th_ph_pm, func=ACTF.Sin,
                                 bias=0.0, scale=0.5)
            # cos(x/2) = Sin(pi/2 - |x|/2): the |.| keeps the argument in
            # the LUT's +-4 range even for |x| up to 5.7
            absx = mid1_pool.tile([pp, 2, T, 22], f32)
            nc.scalar.activation(out=absx[:, :, :, :], in_=th_ph_pm, func=ACTF.Abs)
            nc.scalar.activation(out=half_out2, in_=absx[:, :, :, :],
                                 func=ACTF.Sin, bias=halfpi[:, 0:1], scale=-0.5)

            # TG2 blocks: [ct~ | st~ | sp~ | cp~] where
            #   st~ = s_t*c_t (sin t = 2 st~), ct~ = 0.5 - s_t^2 (cos t = 2 ct~)
            #   sp~ = s_p*c_p,                 cp~ = 0.25 - 0.5 s_p^2
            tg2 = mid1_pool.tile([pp, 4, KT], f32)
            nc.vector.tensor_tensor(
                out=bass.AP(tensor=tg2.tensor, offset=tg2.offset + KT,
                            ap=[tg2.ap[0], [KT, 2], [1, KT]]),
                in0=tgh[:, 0:2], in1=tgh[:, 2:4], op=ALU.mult)
            # squares in place over TGH[0:2] (after the product above)
            nc.scalar.activation(out=tgh[:, 0:2], in_=tgh[:, 0:2],
                                 func=ACTF.Square)
            nc.scalar.activation(out=tg2[:, 0], in_=tgh[:, 0],
                                 func=ACTF.Copy, bias=0.5, scale=-1.0)
            nc.scalar.activation(out=tg2[:, 3], in_=tgh[:, 1],
                                 func=ACTF.Copy, bias=0.25, scale=-0.5)

            # rd4 = 4r dense (ACT)
            rd = mid1_pool.tile([pp, T, 22], f32)
            nc.scalar.activation(out=rd, in_=r4[:, :, 0:66:3],
                                 func=ACTF.Copy, bias=0.0, scale=4.0)
            # Q blocks: [rsin4, x0, x1, x2]; rsin4 = 4r*sp~ = 2 r sin(phi)
            q = mid1_pool.tile([pp, 4, T, 22], f32)
            rd2 = bass.AP(tensor=rd.tensor, offset=rd.offset,
                          ap=[rd.ap[0], [0, 2], [1, KT]])
            # [rsin4, x1] = [4r, 4r] * [sp~, cp~]   (GPSIMD)
            nc.gpsimd.tensor_tensor(
                out=bass.AP(tensor=q.tensor, offset=q.offset,
                            ap=[q.ap[0], [2 * KT, 2], [1, KT]]),
                in0=rd2, in1=tg2[:, 2:4], op=ALU.mult)
            # [x0, x2] = [rsin4, rsin4] * [ct~, st~]   (DVE)
            nc.vector.tensor_tensor(
                out=bass.AP(tensor=q.tensor, offset=q.offset + KT,
                            ap=[q.ap[0], [2 * KT, 2], [1, KT]]),
                in0=bass.AP(tensor=q.tensor, offset=q.offset,
                            ap=[q.ap[0], [0, 2], [1, KT]]),
                in1=tg2[:, 0:2], op=ALU.mult)
            xyz = q[:, 1:4]  # (comp, rep, k) with comps x0, x1, x2

            # ---- assemble scan work buffer W [pp, 3, T, 32] ----
            w = io_pool.tile([pp, 3, T, 32], f32)
            # zero-offset slots (ch 20,28 and 23,31 = IGNORE copies)
            nc.gpsimd.memset(w[:, :, :, 20:29:8], 0.0)
            nc.gpsimd.memset(w[:, :, :, 23:32:8], 0.0)
            # root slots from obs: ch{0,1} <- cols 0..5, ch{6,11} <- 18..20/33..35
            obs_t = mid_pool.tile([pp, 96], f32)
            nc.sync.dma_start(out=obs_t, in_=obs[b0 : b0 + pp, :])
            nc.scalar.copy(
                out=w[:, :, :, 0:2],
                in_=bass.AP(tensor=obs_t.tensor, offset=obs_t.offset,
                            ap=[obs_t.ap[0], [1, 3], [0, T], [3, 2]]),
            )
            nc.scalar.copy(
                out=w[:, :, :, 6:12:5],
                in_=bass.AP(tensor=obs_t.tensor, offset=obs_t.offset + 18,
                            ap=[obs_t.ap[0], [1, 3], [0, T], [15, 2]]),
            )
            # xyz offset slots (7 contiguous runs) on ACT
            for k0, ch0, ln in ASSEM_RUNS:
                nc.scalar.copy(
                    out=w[:, :, :, ch0 : ch0 + ln], in_=xyz[:, :, :, k0 : k0 + ln]
                )
            # correction slots: ch16 = -(k2+k3) -> x13; ch24 = -(k9..k13) -> x13
            nc.vector.tensor_reduce(
                out=w[:, :, :, 16:17], in_=xyz[:, :, :, 2:4],
                axis=mybir.AxisListType.X, op=ALU.add, negate=True)
            nc.vector.tensor_reduce(
                out=w[:, :, :, 24:25], in_=xyz[:, :, :, 9:14],
                axis=mybir.AxisListType.X, op=ALU.add, negate=True)

            # ---- gated scan, all 3 components fused (in place) ----
            w2 = w.rearrange("p c r j -> p (c r j)")
            nc.vector.tensor_tensor_scan(
                out=w2, data0=gate, data1=w2,
                initial=0.0, op0=ALU.mult, op1=ALU.add)

            nc.sync.dma_start(
                out=out[b0 : b0 + pp, :],
                in_=w.rearrange("p c r j -> p (c r j)"),
            )


_CACHE = {}


def _get_nc():
    if "nc" not in _CACHE:
        import concourse.bacc as bacc

        nc = bacc.Bacc("TRN2", target_bir_lowering=False)
        build_kernel(nc, NB)
        nc.compile()
        _CACHE["nc"] = nc
    return _CACHE["nc"]


def _run(in_maps, **kwargs):
    from concourse.bass_utils import run_bass_kernel_spmd

    nc = _get_nc()
    return run_bass_kernel_spmd(nc, in_maps, core_ids=list(range(N_CORES)), **kwargs)


def _make_in_maps(observed_pose, pred_pose):
    obs_last = np.ascontiguousarray(observed_pose[:, -1, :], dtype=np.float32)
    pred = np.ascontiguousarray(pred_pose, dtype=np.float32)
    in_maps = []
    for c in range(N_CORES):
        in_maps.append(
            {
                "pred": np.ascontiguousarray(
                    pred[c * NB : (c + 1) * NB].reshape(NB * T, 66)
                ),
                "obs": obs_last[c * NB : (c + 1) * NB],
            }
        )
    return in_maps


def _assemble_out(results):
    outs = []
    for c in range(N_CORES):
        o = results[c]["out"].reshape(NB, 3, T, 32)
        outs.append(o.transpose(0, 2, 3, 1).reshape(NB, T, 96))
    return np.ascontiguousarray(np.concatenate(outs, axis=0), dtype=np.float32)


def kernel(observed_pose, pred_pose):
    res = _run(_make_in_maps(observed_pose, pred_pose))
    return _assemble_out(res.results)


def kernel_traced(observed_pose, pred_pose, trace_cores=None):
    """Like kernel() but returns (output, BassKernelResults) with an NTFF trace."""
    res = _run(
        _make_in_maps(observed_pose, pred_pose),
        trace=True,
        trace_cores=trace_cores or [0],
    )
    return _assemble_out(res.results), res


# revision 19
# speedup vs baseline: 3.3569x; 1.0144x over previous
"""Trainium2 Bass kernel for the Human3.6M pose postprocess
(spherical->xyz conversion + kinematic-tree accumulation).

Self-contained: hardcodes the problem shapes
  observed_pose (4096, 16, 96) f32, pred_pose (4096, 64, 66) f32
and shards the batch dim across 8 NeuronCores (pure data parallel).

Per-core design (bt-major layout, no transposes, no TensorE):
  - partition p <-> one batch example; free dim = (64 time steps) x channels
  - range reduction to [-pi, pi] without a mod op:
      n  = round(x / 2pi)   via two ACT affine passes (big-constant
                            round-to-nearest trick: +C then -C, C = 1.5*2^23)
      w  = x - 2pi*n        one DVE scalar_tensor_tensor
    theta and phi are processed interleaved (one strided op covers both).
  - sin = Sin(w); cos = Sin(pi/2 - |w|)  (both args within the +-4 LUT range)
  - DVE: muls for spherical->xyz, then ONE gated tensor_tensor_scan per xyz
    component computes the entire 22-edge kinematic tree walk in natural
    output-channel order:  state = gate*state + w;  gate=0 at root channels
    {0,1,6,11} reseeds state from the last observed frame, zero-offset slots
    reproduce the IGNORE copies, and two correction slots (ch16, ch24)
    rewind the state to x[13] across tree branches.
Output leaves the device component-major [nb, 3, 64, 32]; the host
transposes back to (B, T, 96).
"""

import math
import sys

for _p in ("/opt/trn_rl_repo", "/root/.axon_site/_ro/trn_rl_repo"):
    if _p not in sys.path:
        sys.path.insert(0, _p)

import numpy as np

PI = math.pi
BIGC = 1.5 * 2**23  # fp32 round-to-nearest-integer constant
T = 64   # time steps = reps per partition
P = 128  # partitions per tile

N_CORES = 8
B = 4096
NB = B // N_CORES  # batches per core
KERNEL_R = 32      # rows (time steps) per partition

# child-joint order of CONNECT (k index) -> contiguous runs in output-channel
# space: (k_start, ch_start, length)
ASSEM_RUNS = [
    (0, 12, 4),   # k0..3   -> ch12..15  (spine 12,13,14,15)
    (4, 25, 3),   # k4..6   -> ch25..27  (arm 25,26,27)
    (7, 29, 2),   # k7..8   -> ch29..30  (arm 29,30)
    (9, 17, 3),   # k9..11  -> ch17..19  (arm 17,18,19)
    (12, 21, 2),  # k12..13 -> ch21..22  (arm 21,22)
    (14, 2, 4),   # k14..17 -> ch2..5    (leg 2,3,4,5)
    (18, 7, 4),   # k18..21 -> ch7..10   (leg 7,8,9,10)
]


def build_kernel(nc, n_b: int, r: int = 32):
    """Build the postprocess kernel for n_b batch examples on one core."""
    import concourse.tile as tile
    from concourse import mybir

    f32 = mybir.dt.float32
    pred = nc.dram_tensor("pred", [n_b * T, 66], f32, kind="ExternalInput")
    # obs rows pre-duplicated on the host: row j <-> partition j of a tile
    # (each batch example spans 64//r partitions)
    obs = nc.dram_tensor("obs", [n_b * (T // r), 96], f32, kind="ExternalInput")
    out = nc.dram_tensor("out", [n_b, 3 * T * 32], f32, kind="ExternalOutput")

    with tile.TileContext(nc) as tc:
        build_tile_kernel(tc, pred, obs, out, n_b, r)
    return nc


def build_tile_kernel(tc, pred, obs, out, n_b: int, r: int):
    import concourse.bass as bass
    from concourse import mybir

    f32 = mybir.dt.float32
    ALU = mybir.AluOpType
    ACTF = mybir.ActivationFunctionType
    nc = tc.nc
    pp = min(P, n_b * T // r)
    nt = n_b * T // (pp * r)
    hb = r * 32          # out elems per partition per component
    KT = r * 22          # trig elems per half
    bpt = pp * r // T    # batch examples per tile

    # pred rows grouped per tile: partition p holds r consecutive rows
    pred_t = pred.ap().rearrange("(n p q) c -> n p (q c)", p=pp, q=r)

    with (
        tc.tile_pool(name="io", bufs=2) as io_pool,
        tc.tile_pool(name="mid", bufs=2) as mid_pool,
        tc.tile_pool(name="mid1", bufs=2) as mid1_pool,
        tc.tile_pool(name="const", bufs=1) as const_pool,
    ):
        # static gate tile [pp, 3*r*32] (all three components in one scan):
        # 1.0 everywhere, 0.0 at root channels
        gate = const_pool.tile([pp, 3 * hb], f32)
        g4 = gate.rearrange("p (c q j) -> p c q j", c=3, j=32)
        nc.vector.memset(gate, 1.0)
        nc.vector.memset(g4[:, :, :, 0:2], 0.0)       # ch 0, 1
        nc.vector.memset(g4[:, :, :, 6:12:5], 0.0)    # ch 6, 11

        # per-partition constant for Sin biases
        halfpi = const_pool.tile([pp, 1], f32)
        nc.vector.memset(halfpi, PI / 2)

        for i in range(nt):
            raw = io_pool.tile([pp, r * 66], f32)
            nc.sync.dma_start(out=raw, in_=pred_t[i])
            r4 = raw.rearrange("p (q c) -> p q c", c=66)
            # (theta, phi) strided view iterated (pair, rep, joint)
            th_ph_pm = bass.AP(tensor=raw.tensor, offset=raw.offset + 1,
                               ap=[raw.ap[0], [1, 2], [66, r], [3, 22]])

            # ---- trig via half-angle, no range reduction needed ----
            # |x/2| <= 2.85 < pi.  s = Sin(x/2), c = cos(x/2) = Sin(pi/2-|x|/2)
            # sin x = 2sc,  cos x = 1 - 2s^2; scale factors fold downstream.
            # TGH blocks: [s_t | s_p | c_t | c_p] (dense halves)
            tgh = mid1_pool.tile([pp, 4, KT], f32)
            half_out0 = bass.AP(tensor=tgh.tensor, offset=tgh.offset,
                                ap=[tgh.ap[0], [KT, 2], [22, r], [1, 22]])
            half_out2 = bass.AP(tensor=tgh.tensor, offset=tgh.offset + 2 * KT,
                                ap=[tgh.ap[0], [KT, 2], [22, r], [1, 22]])
            nc.scalar.activation(out=half_out0, in_=th_ph_pm, func=ACTF.Sin,
                                 bias=0.0, scale=0.5)
            absx = mid1_pool.tile([pp, 2, r, 22], f32)
            nc.scalar.activation(out=absx[:, :, :, :], in_=th_ph_pm,
                                 func=ACTF.Abs)
            nc.scalar.activation(out=half_out2, in_=absx[:, :, :, :],
                                 func=ACTF.Sin, bias=halfpi[:, 0:1], scale=-0.5)

            # TG2 blocks: [ct~ | st~ | sp~ | cp~]:
            #   st~ = s_t c_t (sin t = 2 st~),  ct~ = 0.5 - s_t^2
            #   sp~ = s_p c_p,                  cp~ = 0.25 - 0.5 s_p^2
            tg2 = mid1_pool.tile([pp, 4, KT], f32)
            nc.vector.tensor_tensor(
                out=bass.AP(tensor=tg2.tensor, offset=tg2.offset + KT,
                            ap=[tg2.ap[0], [KT, 2], [1, KT]]),
                in0=tgh[:, 0:2], in1=tgh[:, 2:4], op=ALU.mult)
            nc.scalar.activation(out=tgh[:, 0:2], in_=tgh[:, 0:2],
                                 func=ACTF.Square)
            nc.scalar.activation(out=tg2[:, 0], in_=tgh[:, 0],
                                 func=ACTF.Copy, bias=0.5, scale=-1.0)
            nc.scalar.activation(out=tg2[:, 3], in_=tgh[:, 1],
                                 func=ACTF.Copy, bias=0.25, scale=-0.5)

            # rd4 = 4r dense (ACT); Q blocks: [rsin4, x0, x1, x2]
            rd = mid1_pool.tile([pp, r, 22], f32)
            nc.scalar.activation(out=rd, in_=r4[:, :, 0:66:3],
                                 func=ACTF.Copy, bias=0.0, scale=4.0)
            q = mid1_pool.tile([pp, 4, r, 22], f32)
            rd2 = bass.AP(tensor=rd.tensor, offset=rd.offset,
                          ap=[rd.ap[0], [0, 2], [1, KT]])
            # [rsin4, x1] = [4r, 4r] * [sp~, cp~]   (GPSIMD)
            nc.gpsimd.tensor_tensor(
                out=bass.AP(tensor=q.tensor, offset=q.offset,
                            ap=[q.ap[0], [2 * KT, 2], [1, KT]]),
                in0=rd2, in1=tg2[:, 2:4], op=ALU.mult)
            # [x0, x2] = [rsin4, rsin4] * [ct~, st~]   (DVE)
            nc.vector.tensor_tensor(
                out=bass.AP(tensor=q.tensor, offset=q.offset + KT,
                            ap=[q.ap[0], [2 * KT, 2], [1, KT]]),
                in0=bass.AP(tensor=q.tensor, offset=q.offset,
                            ap=[q.ap[0], [0, 2], [1, KT]]),
                in1=tg2[:, 0:2], op=ALU.mult)
            xyz = q[:, 1:4]  # (comp, rep, k) with comps x0, x1, x2

            # ---- assemble scan work buffer W [pp, 3, r, 32] ----
            w = io_pool.tile([pp, 3, r, 32], f32)
            nc.gpsimd.memset(w[:, :, :, 20:29:8], 0.0)
            nc.gpsimd.memset(w[:, :, :, 23:32:8], 0.0)
            # root slots from obs (host-duplicated rows: one row per partition)
            obs_t = mid_pool.tile([pp, 96], f32)
            nc.sync.dma_start(out=obs_t, in_=obs[i * pp : (i + 1) * pp, :])
            nc.scalar.copy(
                out=w[:, :, :, 0:2],
                in_=bass.AP(tensor=obs_t.tensor, offset=obs_t.offset,
                            ap=[obs_t.ap[0], [1, 3], [0, r], [3, 2]]),
            )
            nc.scalar.copy(
                out=w[:, :, :, 6:12:5],
                in_=bass.AP(tensor=obs_t.tensor, offset=obs_t.offset + 18,
                            ap=[obs_t.ap[0], [1, 3], [0, r], [15, 2]]),
            )
            # xyz offset slots (7 contiguous runs) on ACT
            for k0, ch0, ln in ASSEM_RUNS:
                nc.scalar.copy(
                    out=w[:, :, :, ch0 : ch0 + ln], in_=xyz[:, :, :, k0 : k0 + ln]
                )
            # correction slots: ch16 = -(k2+k3); ch24 = -(k9..k13)
            nc.vector.tensor_reduce(
                out=w[:, :, :, 16:17], in_=xyz[:, :, :, 2:4],
                axis=mybir.AxisListType.X, op=ALU.add, negate=True)
            nc.vector.tensor_reduce(
                out=w[:, :, :, 24:25], in_=xyz[:, :, :, 9:14],
                axis=mybir.AxisListType.X, op=ALU.add, negate=True)

            # ---- gated scan, all 3 components fused, out of place ----
            ot = io_pool.tile([pp, 3, hb], f32)
            nc.vector.tensor_tensor_scan(
                out=ot.rearrange("p c f -> p (c f)"), data0=gate,
                data1=w.rearrange("p c q j -> p (c q j)"),
                initial=0.0, op0=ALU.mult, op1=ALU.add)

            # out DMA per component: DRAM [b, c*2048 + t*32 + ch] with
            # b = i*bpt + p // (T//r), t = (p % (T//r))*r + rep
            for c in range(3):
                nc.sync.dma_start(
                    out=bass.AP(
                        tensor=out,
                        offset=(i * bpt) * (3 * T * 32) + c * (T * 32),
                        ap=[[3 * T * 32, bpt], [hb, T // r], [1, hb]],
                    ),
                    in_=ot[:, c],
                )


_CACHE = {}


def _get_nc():
    if "nc" not in _CACHE:
        import concourse.bacc as bacc

        nc = bacc.Bacc("TRN2", target_bir_lowering=False)
        build_kernel(nc, NB, r=KERNEL_R)
        nc.compile()
        _CACHE["nc"] = nc
    return _CACHE["nc"]


def _run(in_maps, **kwargs):
    from concourse.bass_utils import run_bass_kernel_spmd

    nc = _get_nc()
    return run_bass_kernel_spmd(nc, in_maps, core_ids=list(range(N_CORES)), **kwargs)


def _make_in_maps(observed_pose, pred_pose):
    obs_last = np.ascontiguousarray(observed_pose[:, -1, :], dtype=np.float32)
    # one obs row per tile-partition: duplicate each row T//r times
    obs_dup = np.repeat(obs_last, T // KERNEL_R, axis=0)
    pred = np.ascontiguousarray(pred_pose, dtype=np.float32)
    dup = T // KERNEL_R
    in_maps = []
    for c in range(N_CORES):
        in_maps.append(
            {
                "pred": np.ascontiguousarray(
                    pred[c * NB : (c + 1) * NB].reshape(NB * T, 66)
                ),
                "obs": np.ascontiguousarray(obs_dup[c * NB * dup : (c + 1) * NB * dup]),
            }
        )
    return in_maps


def _assemble_out(results):
    outs = []
    for c in range(N_CORES):
        o = results[c]["out"].reshape(NB, 3, T, 32)
        outs.append(o.transpose(0, 2, 3, 1).reshape(NB, T, 96))
    return np.ascontiguousarray(np.concatenate(outs, axis=0), dtype=np.float32)


def kernel(observed_pose, pred_pose):
    res = _run(_make_in_maps(observed_pose, pred_pose))
    return _assemble_out(res.results)


def kernel_traced(observed_pose, pred_pose, trace_cores=None):
    """Like kernel() but returns (output, BassKernelResults) with an NTFF trace."""
    res = _run(
        _make_in_maps(observed_pose, pred_pose),
        trace=True,
        trace_cores=trace_cores or [0],
    )
    return _assemble_out(res.results), res


# revision 20
# speedup vs baseline: 3.4743x; 1.0350x over previous
"""Trainium2 Bass kernel for the Human3.6M pose postprocess
(spherical->xyz conversion + kinematic-tree accumulation).

Self-contained: hardcodes the problem shapes
  observed_pose (4096, 16, 96) f32, pred_pose (4096, 64, 66) f32
and shards the batch dim across 8 NeuronCores (pure data parallel).

Per-core design (bt-major layout, no transposes, no TensorE):
  - partition p <-> one batch example; free dim = (64 time steps) x channels
  - range reduction to [-pi, pi] without a mod op:
      n  = round(x / 2pi)   via two ACT affine passes (big-constant
                            round-to-nearest trick: +C then -C, C = 1.5*2^23)
      w  = x - 2pi*n        one DVE scalar_tensor_tensor
    theta and phi are processed interleaved (one strided op covers both).
  - sin = Sin(w); cos = Sin(pi/2 - |w|)  (both args within the +-4 LUT range)
  - DVE: muls for spherical->xyz, then ONE gated tensor_tensor_scan per xyz
    component computes the entire 22-edge kinematic tree walk in natural
    output-channel order:  state = gate*state + w;  gate=0 at root channels
    {0,1,6,11} reseeds state from the last observed frame, zero-offset slots
    reproduce the IGNORE copies, and two correction slots (ch16, ch24)
    rewind the state to x[13] across tree branches.
Output leaves the device component-major [nb, 3, 64, 32]; the host
transposes back to (B, T, 96).
"""

import math
import sys

for _p in ("/opt/trn_rl_repo", "/root/.axon_site/_ro/trn_rl_repo"):
    if _p not in sys.path:
        sys.path.insert(0, _p)

import numpy as np

PI = math.pi
BIGC = 1.5 * 2**23  # fp32 round-to-nearest-integer constant
T = 64   # time steps = reps per partition
P = 128  # partitions per tile

N_CORES = 8
B = 4096
NB = B // N_CORES  # batches per core
KERNEL_R = 32      # rows (time steps) per partition

# child-joint order of CONNECT (k index) -> contiguous runs in output-channel
# space: (k_start, ch_start, length)
ASSEM_RUNS = [
    (0, 12, 4),   # k0..3   -> ch12..15  (spine 12,13,14,15)
    (4, 25, 3),   # k4..6   -> ch25..27  (arm 25,26,27)
    (7, 29, 2),   # k7..8   -> ch29..30  (arm 29,30)
    (9, 17, 3),   # k9..11  -> ch17..19  (arm 17,18,19)
    (12, 21, 2),  # k12..13 -> ch21..22  (arm 21,22)
    (14, 2, 4),   # k14..17 -> ch2..5    (leg 2,3,4,5)
    (18, 7, 4),   # k18..21 -> ch7..10   (leg 7,8,9,10)
]


def build_kernel(nc, n_b: int, r: int = 32):
    """Build the postprocess kernel for n_b batch examples on one core."""
    import concourse.tile as tile
    from concourse import mybir

    f32 = mybir.dt.float32
    pred = nc.dram_tensor("pred", [n_b * T, 66], f32, kind="ExternalInput")
    # obs rows pre-duplicated on the host: row j <-> partition j of a tile
    # (each batch example spans 64//r partitions)
    obs = nc.dram_tensor("obs", [n_b * (T // r), 96], f32, kind="ExternalInput")
    out = nc.dram_tensor("out", [n_b, 3 * T * 32], f32, kind="ExternalOutput")

    with tile.TileContext(nc) as tc:
        build_tile_kernel(tc, pred, obs, out, n_b, r)
    return nc


def build_tile_kernel(tc, pred, obs, out, n_b: int, r: int):
    import concourse.bass as bass
    from concourse import mybir

    f32 = mybir.dt.float32
    ALU = mybir.AluOpType
    ACTF = mybir.ActivationFunctionType
    nc = tc.nc
    pp = min(P, n_b * T // r)
    nt = n_b * T // (pp * r)
    hb = r * 32          # out elems per partition per component
    KT = r * 22          # trig elems per half
    bpt = pp * r // T    # batch examples per tile

    # pred rows grouped per tile: partition p holds r consecutive rows
    pred_t = pred.ap().rearrange("(n p q) c -> n p (q c)", p=pp, q=r)

    with (
        tc.tile_pool(name="io", bufs=2) as io_pool,
        tc.tile_pool(name="mid", bufs=2) as mid_pool,
        tc.tile_pool(name="mid1", bufs=2) as mid1_pool,
        tc.tile_pool(name="const", bufs=1) as const_pool,
    ):
        # static gate tile [pp, 3*r*32] (all three components in one scan):
        # 1.0 everywhere, 0.0 at root channels
        gate = const_pool.tile([pp, 3 * hb], f32)
        g4 = gate.rearrange("p (c q j) -> p c q j", c=3, j=32)
        nc.vector.memset(gate, 1.0)
        nc.vector.memset(g4[:, :, :, 0:2], 0.0)       # ch 0, 1
        nc.vector.memset(g4[:, :, :, 6:12:5], 0.0)    # ch 6, 11

        # per-partition constant for Sin biases
        halfpi = const_pool.tile([pp, 1], f32)
        nc.vector.memset(halfpi, PI / 2)

        for i in range(nt):
            raw = io_pool.tile([pp, r * 66], f32)
            nc.sync.dma_start(out=raw, in_=pred_t[i])
            r4 = raw.rearrange("p (q c) -> p q c", c=66)
            # (theta, phi) strided view iterated (pair, rep, joint)
            th_ph_pm = bass.AP(tensor=raw.tensor, offset=raw.offset + 1,
                               ap=[raw.ap[0], [1, 2], [66, r], [3, 22]])

            # ---- trig via half-angle, no range reduction needed ----
            # |x/2| <= 2.85 < pi.  s = Sin(x/2), c = cos(x/2) = Sin(pi/2-|x|/2)
            # sin x = 2sc,  cos x = 1 - 2s^2; scale factors fold downstream.
            # TGH blocks: [s_t | s_p | c_t | c_p] (dense halves)
            tgh = mid1_pool.tile([pp, 4, KT], f32)
            half_out0 = bass.AP(tensor=tgh.tensor, offset=tgh.offset,
                                ap=[tgh.ap[0], [KT, 2], [22, r], [1, 22]])
            half_out2 = bass.AP(tensor=tgh.tensor, offset=tgh.offset + 2 * KT,
                                ap=[tgh.ap[0], [KT, 2], [22, r], [1, 22]])
            nc.scalar.activation(out=half_out0, in_=th_ph_pm, func=ACTF.Sin,
                                 bias=0.0, scale=0.5)
            absx = mid1_pool.tile([pp, 2, r, 22], f32)
            nc.scalar.activation(out=absx[:, :, :, :], in_=th_ph_pm,
                                 func=ACTF.Abs)
            nc.scalar.activation(out=half_out2, in_=absx[:, :, :, :],
                                 func=ACTF.Sin, bias=halfpi[:, 0:1], scale=-0.5)

            # TG2 blocks: [ct~ | st~ | sp~ | cp~]:
            #   st~ = s_t c_t (sin t = 2 st~),  ct~ = 0.5 - s_t^2
            #   sp~ = s_p c_p,                  cp~ = 0.25 - 0.5 s_p^2
            tg2 = mid1_pool.tile([pp, 4, r, 22], f32)
            nc.vector.tensor_tensor(
                out=bass.AP(tensor=tg2.tensor, offset=tg2.offset + KT,
                            ap=[tg2.ap[0], [KT, 2], [22, r], [1, 22]]),
                in0=tgh[:, 0:2], in1=tgh[:, 2:4], op=ALU.mult)
            nc.scalar.activation(out=tgh[:, 0:2], in_=tgh[:, 0:2],
                                 func=ACTF.Square)
            nc.scalar.activation(out=tg2[:, 0], in_=tgh[:, 0],
                                 func=ACTF.Copy, bias=0.5, scale=-1.0)
            nc.scalar.activation(out=tg2[:, 3], in_=tgh[:, 1],
                                 func=ACTF.Copy, bias=0.25, scale=-0.5)

            # rd4 = 4r dense (GPSIMD copy of pre-scaled? no: ACT scale)
            rd = mid1_pool.tile([pp, r, 22], f32)
            nc.scalar.activation(out=rd, in_=r4[:, :, 0:66:3],
                                 func=ACTF.Copy, bias=0.0, scale=4.0)
            # rsin4 = 4r * sp~ = 2 r sin(phi)   (GPSIMD)
            rs = mid1_pool.tile([pp, r, 22], f32)
            nc.gpsimd.tensor_tensor(out=rs, in0=rd, in1=tg2[:, 2], op=ALU.mult)

            # ---- W [pp, 3, r, 32]: muls write x0/x1/x2 straight into
            # their output-channel slots (no assembly copies) ----
            w = io_pool.tile([pp, 3, r, 32], f32)
            nc.gpsimd.memset(w[:, :, :, 20:29:8], 0.0)
            nc.gpsimd.memset(w[:, :, :, 23:32:8], 0.0)
            hb32 = r * 32
            for k0, ch0, ln in ASSEM_RUNS:
                # [x0, x2] = [rsin4, rsin4] * [ct~, st~]  -> comps 0, 2  (DVE)
                nc.vector.tensor_tensor(
                    out=bass.AP(tensor=w.tensor, offset=w.offset + ch0,
                                ap=[w.ap[0], [2 * hb32, 2], [32, r], [1, ln]]),
                    in0=bass.AP(tensor=rs.tensor, offset=rs.offset + k0,
                                ap=[rs.ap[0], [0, 2], [22, r], [1, ln]]),
                    in1=bass.AP(tensor=tg2.tensor, offset=tg2.offset + k0,
                                ap=[tg2.ap[0], [KT, 2], [22, r], [1, ln]]),
                    op=ALU.mult)
                # x1 = 4r * cp~ = r cos(phi)  -> comp 1  (GPSIMD)
                nc.gpsimd.tensor_tensor(
                    out=w[:, 1, :, ch0 : ch0 + ln],
                    in0=rd[:, :, k0 : k0 + ln],
                    in1=tg2[:, 3, :, k0 : k0 + ln],
                    op=ALU.mult)

            # root slots from obs (host-duplicated rows: one row per partition)
            obs_t = mid_pool.tile([pp, 96], f32)
            nc.sync.dma_start(out=obs_t, in_=obs[i * pp : (i + 1) * pp, :])
            nc.scalar.copy(
                out=w[:, :, :, 0:2],
                in_=bass.AP(tensor=obs_t.tensor, offset=obs_t.offset,
                            ap=[obs_t.ap[0], [1, 3], [0, r], [3, 2]]),
            )
            nc.scalar.copy(
                out=w[:, :, :, 6:12:5],
                in_=bass.AP(tensor=obs_t.tensor, offset=obs_t.offset + 18,
                            ap=[obs_t.ap[0], [1, 3], [0, r], [15, 2]]),
            )
            # corrections, reading W itself:
            #   ch16 = -(W14+W15) = -(k2+k3);  ch24 = -(W17..W22) (W20 = 0)
            nc.vector.tensor_reduce(
                out=w[:, :, :, 16:17], in_=w[:, :, :, 14:16],
                axis=mybir.AxisListType.X, op=ALU.add, negate=True)
            nc.vector.tensor_reduce(
                out=w[:, :, :, 24:25], in_=w[:, :, :, 17:23],
                axis=mybir.AxisListType.X, op=ALU.add, negate=True)

            # ---- gated scan, all 3 components fused, out of place ----
            ot = io_pool.tile([pp, 3, hb], f32)
            nc.vector.tensor_tensor_scan(
                out=ot.rearrange("p c f -> p (c f)"), data0=gate,
                data1=w.rearrange("p c q j -> p (c q j)"),
                initial=0.0, op0=ALU.mult, op1=ALU.add)

            # out DMA per component: DRAM [b, c*2048 + t*32 + ch] with
            # b = i*bpt + p // (T//r), t = (p % (T//r))*r + rep
            for c in range(3):
                nc.sync.dma_start(
                    out=bass.AP(
                        tensor=out,
                        offset=(i * bpt) * (3 * T * 32) + c * (T * 32),
                        ap=[[3 * T * 32, bpt], [hb, T // r], [1, hb]],
                    ),
                    in_=ot[:, c],
                )


_CACHE = {}


def _get_nc():
    if "nc" not in _CACHE:
        import concourse.bacc as bacc

        nc = bacc.Bacc("TRN2", target_bir_lowering=False)
        build_kernel(nc, NB, r=KERNEL_R)
        nc.compile()
        _CACHE["nc"] = nc
    return _CACHE["nc"]


def _run(in_maps, **kwargs):
    from concourse.bass_utils import run_bass_kernel_spmd

    nc = _get_nc()
    return run_bass_kernel_spmd(nc, in_maps, core_ids=list(range(N_CORES)), **kwargs)


def _make_in_maps(observed_pose, pred_pose):
    obs_last = np.ascontiguousarray(observed_pose[:, -1, :], dtype=np.float32)
    # one obs row per tile-partition: duplicate each row T//r times
    obs_dup = np.repeat(obs_last, T // KERNEL_R, axis=0)
    pred = np.ascontiguousarray(pred_pose, dtype=np.float32)
    dup = T // KERNEL_R
    in_maps = []
    for c in range(N_CORES):
        in_maps.append(
            {
                "pred": np.ascontiguousarray(
                    pred[c * NB : (c + 1) * NB].reshape(NB * T, 66)
                ),
                "obs": np.ascontiguousarray(obs_dup[c * NB * dup : (c + 1) * NB * dup]),
            }
        )
    return in_maps


def _assemble_out(results):
    outs = []
    for c in range(N_CORES):
        o = results[c]["out"].reshape(NB, 3, T, 32)
        outs.append(o.transpose(0, 2, 3, 1).reshape(NB, T, 96))
    return np.ascontiguousarray(np.concatenate(outs, axis=0), dtype=np.float32)


def kernel(observed_pose, pred_pose):
    res = _run(_make_in_maps(observed_pose, pred_pose))
    return _assemble_out(res.results)


def kernel_traced(observed_pose, pred_pose, trace_cores=None):
    """Like kernel() but returns (output, BassKernelResults) with an NTFF trace."""
    res = _run(
        _make_in_maps(observed_pose, pred_pose),
        trace=True,
        trace_cores=trace_cores or [0],
    )
    return _assemble_out(res.results), res


# revision 22
# speedup vs baseline: 3.6375x; 1.0470x over previous
"""Trainium2 Bass kernel for the Human3.6M pose postprocess
(spherical->xyz conversion + kinematic-tree accumulation).

Self-contained: hardcodes the problem shapes
  observed_pose (4096, 16, 96) f32, pred_pose (4096, 64, 66) f32
and shards the batch dim across 8 NeuronCores (pure data parallel).

Per-core design (bt-major layout, no transposes, no TensorE):
  - partition p <-> one batch example; free dim = (64 time steps) x channels
  - range reduction to [-pi, pi] without a mod op:
      n  = round(x / 2pi)   via two ACT affine passes (big-constant
                            round-to-nearest trick: +C then -C, C = 1.5*2^23)
      w  = x - 2pi*n        one DVE scalar_tensor_tensor
    theta and phi are processed interleaved (one strided op covers both).
  - sin = Sin(w); cos = Sin(pi/2 - |w|)  (both args within the +-4 LUT range)
  - DVE: muls for spherical->xyz, then ONE gated tensor_tensor_scan per xyz
    component computes the entire 22-edge kinematic tree walk in natural
    output-channel order:  state = gate*state + w;  gate=0 at root channels
    {0,1,6,11} reseeds state from the last observed frame, zero-offset slots
    reproduce the IGNORE copies, and two correction slots (ch16, ch24)
    rewind the state to x[13] across tree branches.
Output leaves the device component-major [nb, 3, 64, 32]; the host
transposes back to (B, T, 96).
"""

import math
import sys

for _p in ("/opt/trn_rl_repo", "/root/.axon_site/_ro/trn_rl_repo"):
    if _p not in sys.path:
        sys.path.insert(0, _p)

import numpy as np

PI = math.pi
BIGC = 1.5 * 2**23  # fp32 round-to-nearest-integer constant
T = 64   # time steps = reps per partition
P = 128  # partitions per tile

N_CORES = 8
B = 4096
NB = B // N_CORES  # batches per core
KERNEL_R = 32      # rows (time steps) per partition

# child-joint order of CONNECT (k index) -> contiguous runs in output-channel
# space: (k_start, ch_start, length)
ASSEM_RUNS = [
    (0, 12, 4),   # k0..3   -> ch12..15  (spine 12,13,14,15)
    (4, 25, 3),   # k4..6   -> ch25..27  (arm 25,26,27)
    (7, 29, 2),   # k7..8   -> ch29..30  (arm 29,30)
    (9, 17, 3),   # k9..11  -> ch17..19  (arm 17,18,19)
    (12, 21, 2),  # k12..13 -> ch21..22  (arm 21,22)
    (14, 2, 4),   # k14..17 -> ch2..5    (leg 2,3,4,5)
    (18, 7, 4),   # k18..21 -> ch7..10   (leg 7,8,9,10)
]


def build_kernel(nc, n_b: int, r: int = 32):
    """Build the postprocess kernel for n_b batch examples on one core."""
    import concourse.tile as tile
    from concourse import mybir

    f32 = mybir.dt.float32
    pred = nc.dram_tensor("pred", [n_b * T, 66], f32, kind="ExternalInput")
    # obs rows pre-duplicated on the host: row j <-> partition j of a tile
    # (each batch example spans 64//r partitions)
    obs = nc.dram_tensor("obs", [n_b * (T // r), 96], f32, kind="ExternalInput")
    out = nc.dram_tensor("out", [n_b, 3 * T * 32], f32, kind="ExternalOutput")

    with tile.TileContext(nc) as tc:
        build_tile_kernel(tc, pred, obs, out, n_b, r)
    return nc


def build_tile_kernel(tc, pred, obs, out, n_b: int, r: int):
    import concourse.bass as bass
    from concourse import mybir

    f32 = mybir.dt.float32
    ALU = mybir.AluOpType
    ACTF = mybir.ActivationFunctionType
    nc = tc.nc
    pp = min(P, n_b * T // r)
    nt = n_b * T // (pp * r)
    hb = r * 32          # out elems per partition per component
    KT = r * 22          # trig elems per half
    bpt = pp * r // T    # batch examples per tile

    # pred rows grouped per tile: partition p holds r consecutive rows
    pred_t = pred.ap().rearrange("(n p q) c -> n p (q c)", p=pp, q=r)

    with (
        tc.tile_pool(name="io", bufs=3) as io_pool,
        tc.tile_pool(name="mid", bufs=2) as mid_pool,
        tc.tile_pool(name="mid1", bufs=2) as mid1_pool,
        tc.tile_pool(name="const", bufs=1) as const_pool,
    ):
        # static gate tile [pp, 3*r*32] (all three components in one scan):
        # 1.0 everywhere, 0.0 at root channels
        gate = const_pool.tile([pp, 3 * hb], f32)
        g4 = gate.rearrange("p (c q j) -> p c q j", c=3, j=32)
        nc.vector.memset(gate, 1.0)
        nc.vector.memset(g4[:, :, :, 0:2], 0.0)       # ch 0, 1
        nc.vector.memset(g4[:, :, :, 6:12:5], 0.0)    # ch 6, 11

        # per-partition constant for Sin biases
        halfpi = const_pool.tile([pp, 1], f32)
        nc.vector.memset(halfpi, PI / 2)

        for i in range(nt):
            raw = io_pool.tile([pp, r * 66], f32)
            nc.sync.dma_start(out=raw, in_=pred_t[i])
            r4 = raw.rearrange("p (q c) -> p q c", c=66)
            # (theta, phi) strided view iterated (pair, rep, joint)
            th_ph_pm = bass.AP(tensor=raw.tensor, offset=raw.offset + 1,
                               ap=[raw.ap[0], [1, 2], [66, r], [3, 22]])

            # ---- trig via half-angle, no range reduction needed ----
            # |x/2| <= 2.85 < pi.  s = Sin(x/2), c = cos(x/2) = Sin(pi/2-|x|/2)
            # sin x = 2sc,  cos x = 1 - 2s^2; scale factors fold downstream.
            # TGH blocks: [s_t | s_p | c_t | c_p] (dense halves)
            tgh = mid1_pool.tile([pp, 4, KT], f32)
            half_out0 = bass.AP(tensor=tgh.tensor, offset=tgh.offset,
                                ap=[tgh.ap[0], [KT, 2], [22, r], [1, 22]])
            half_out2 = bass.AP(tensor=tgh.tensor, offset=tgh.offset + 2 * KT,
                                ap=[tgh.ap[0], [KT, 2], [22, r], [1, 22]])
            nc.scalar.activation(out=half_out0, in_=th_ph_pm, func=ACTF.Sin,
                                 bias=0.0, scale=0.5)
            absx = mid1_pool.tile([pp, 2, r, 22], f32)
            nc.scalar.activation(out=absx[:, :, :, :], in_=th_ph_pm,
                                 func=ACTF.Abs)
            nc.scalar.activation(out=half_out2, in_=absx[:, :, :, :],
                                 func=ACTF.Sin, bias=halfpi[:, 0:1], scale=-0.5)

            # TG2 blocks: [ct~ | st~ | sp~ | cp~]:
            #   st~ = s_t c_t (sin t = 2 st~),  ct~ = 0.5 - s_t^2
            #   sp~ = s_p c_p,                  cp~ = 0.25 - 0.5 s_p^2
            tg2 = mid1_pool.tile([pp, 4, r, 22], f32)
            nc.vector.tensor_tensor(
                out=bass.AP(tensor=tg2.tensor, offset=tg2.offset + KT,
                            ap=[tg2.ap[0], [KT, 2], [22, r], [1, 22]]),
                in0=tgh[:, 0:2], in1=tgh[:, 2:4], op=ALU.mult)
            sqt = mid1_pool.tile([pp, 2, KT], f32)
            nc.scalar.activation(out=sqt, in_=tgh[:, 0:2], func=ACTF.Square)
            nc.scalar.activation(out=tg2[:, 0], in_=sqt[:, 0].rearrange(
                "p (q k) -> p q k", k=22), func=ACTF.Copy, bias=0.5, scale=-1.0)
            nc.scalar.activation(out=tg2[:, 3], in_=sqt[:, 1].rearrange(
                "p (q k) -> p q k", k=22), func=ACTF.Copy, bias=0.25, scale=-0.5)

            # rd4 = 4r dense (GPSIMD copy of pre-scaled? no: ACT scale)
            rd = mid1_pool.tile([pp, r, 22], f32)
            nc.scalar.activation(out=rd, in_=r4[:, :, 0:66:3],
                                 func=ACTF.Copy, bias=0.0, scale=4.0)
            # rsin4 = 4r * sp~ = 2 r sin(phi)   (GPSIMD)
            rs = mid1_pool.tile([pp, r, 22], f32)
            nc.gpsimd.tensor_tensor(out=rs, in0=rd, in1=tg2[:, 2], op=ALU.mult)

            # ---- W [pp, 3, r, 32]: muls write x0/x1/x2 straight into
            # their output-channel slots (no assembly copies) ----
            w = io_pool.tile([pp, 3, r, 32], f32)
            nc.gpsimd.memset(w[:, :, :, 20:29:8], 0.0)
            nc.gpsimd.memset(w[:, :, :, 23:32:8], 0.0)
            hb32 = r * 32
            for k0, ch0, ln in ASSEM_RUNS:
                # [x0, x2] = [rsin4, rsin4] * [ct~, st~]  -> comps 0, 2  (DVE)
                nc.vector.tensor_tensor(
                    out=bass.AP(tensor=w.tensor, offset=w.offset + ch0,
                                ap=[w.ap[0], [2 * hb32, 2], [32, r], [1, ln]]),
                    in0=bass.AP(tensor=rs.tensor, offset=rs.offset + k0,
                                ap=[rs.ap[0], [0, 2], [22, r], [1, ln]]),
                    in1=bass.AP(tensor=tg2.tensor, offset=tg2.offset + k0,
                                ap=[tg2.ap[0], [KT, 2], [22, r], [1, ln]]),
                    op=ALU.mult)
                # x1 = 4r * cp~ = r cos(phi)  -> comp 1  (GPSIMD)
                nc.gpsimd.tensor_tensor(
                    out=w[:, 1, :, ch0 : ch0 + ln],
                    in0=rd[:, :, k0 : k0 + ln],
                    in1=tg2[:, 3, :, k0 : k0 + ln],
                    op=ALU.mult)

            # root slots from obs (host-duplicated rows: one row per partition)
            obs_t = mid_pool.tile([pp, 96], f32)
            nc.sync.dma_start(out=obs_t, in_=obs[i * pp : (i + 1) * pp, :])
            nc.scalar.copy(
                out=w[:, :, :, 0:2],
                in_=bass.AP(tensor=obs_t.tensor, offset=obs_t.offset,
                            ap=[obs_t.ap[0], [1, 3], [0, r], [3, 2]]),
            )
            nc.scalar.copy(
                out=w[:, :, :, 6:12:5],
                in_=bass.AP(tensor=obs_t.tensor, offset=obs_t.offset + 18,
                            ap=[obs_t.ap[0], [1, 3], [0, r], [15, 2]]),
            )
            # corrections, reading W itself:
            #   ch16 = -(W14+W15) = -(k2+k3);  ch24 = -(W17..W22) (W20 = 0)
            nc.vector.tensor_reduce(
                out=w[:, :, :, 16:17], in_=w[:, :, :, 14:16],
                axis=mybir.AxisListType.X, op=ALU.add, negate=True)
            nc.vector.tensor_reduce(
                out=w[:, :, :, 24:25], in_=w[:, :, :, 17:23],
                axis=mybir.AxisListType.X, op=ALU.add, negate=True)

            # ---- gated scan, all 3 components fused, out of place ----
            ot = io_pool.tile([pp, 3, hb], f32)
            nc.vector.tensor_tensor_scan(
                out=ot.rearrange("p c f -> p (c f)"), data0=gate,
                data1=w.rearrange("p c q j -> p (c q j)"),
                initial=0.0, op0=ALU.mult, op1=ALU.add)

            # out DMA per component: DRAM [b, c*2048 + t*32 + ch] with
            # b = i*bpt + p // (T//r), t = (p % (T//r))*r + rep
            for c in range(3):
                nc.sync.dma_start(
                    out=bass.AP(
                        tensor=out,
                        offset=(i * bpt) * (3 * T * 32) + c * (T * 32),
                        ap=[[3 * T * 32, bpt], [hb, T // r], [1, hb]],
                    ),
                    in_=ot[:, c],
                )


_CACHE = {}


def _get_nc():
    if "nc" not in _CACHE:
        import concourse.bacc as bacc

        nc = bacc.Bacc("TRN2", target_bir_lowering=False)
        build_kernel(nc, NB, r=KERNEL_R)
        nc.compile()
        _CACHE["nc"] = nc
    return _CACHE["nc"]


def _run(in_maps, **kwargs):
    from concourse.bass_utils import run_bass_kernel_spmd

    nc = _get_nc()
    return run_bass_kernel_spmd(nc, in_maps, core_ids=list(range(N_CORES)), **kwargs)


def _make_in_maps(observed_pose, pred_pose):
    obs_last = np.ascontiguousarray(observed_pose[:, -1, :], dtype=np.float32)
    # one obs row per tile-partition: duplicate each row T//r times
    obs_dup = np.repeat(obs_last, T // KERNEL_R, axis=0)
    pred = np.ascontiguousarray(pred_pose, dtype=np.float32)
    dup = T // KERNEL_R
    in_maps = []
    for c in range(N_CORES):
        in_maps.append(
            {
                "pred": np.ascontiguousarray(
                    pred[c * NB : (c + 1) * NB].reshape(NB * T, 66)
                ),
                "obs": np.ascontiguousarray(obs_dup[c * NB * dup : (c + 1) * NB * dup]),
            }
        )
    return in_maps


def _assemble_out(results):
    outs = []
    for c in range(N_CORES):
        o = results[c]["out"].reshape(NB, 3, T, 32)
        outs.append(o.transpose(0, 2, 3, 1).reshape(NB, T, 96))
    return np.ascontiguousarray(np.concatenate(outs, axis=0), dtype=np.float32)


def kernel(observed_pose, pred_pose):
    res = _run(_make_in_maps(observed_pose, pred_pose))
    return _assemble_out(res.results)


def kernel_traced(observed_pose, pred_pose, trace_cores=None):
    """Like kernel() but returns (output, BassKernelResults) with an NTFF trace."""
    res = _run(
        _make_in_maps(observed_pose, pred_pose),
        trace=True,
        trace_cores=trace_cores or [0],
    )
    return _assemble_out(res.results), res


# revision 23
# speedup vs baseline: 3.6405x; 1.0008x over previous
"""Trainium2 Bass kernel for the Human3.6M pose postprocess
(spherical->xyz conversion + kinematic-tree accumulation).

Self-contained: hardcodes the problem shapes
  observed_pose (4096, 16, 96) f32, pred_pose (4096, 64, 66) f32
and shards the batch dim across 8 NeuronCores (pure data parallel,
512 examples per core).

Per-core design (bt-major layout, no transposes, no TensorE):
  - partition p <-> 32 consecutive time rows of one batch example;
    free dim holds (time reps) x channels.  All DMA is >=4KB contiguous
    per partition.
  - trig via half-angle identities, so no range reduction is needed
    (|x/2| <= 2.85 < pi fits the Sin LUT):
      s = Sin(x/2), c = cos(x/2) = Sin(pi/2 - |x|/2)
      sin x = 2sc,  cos x = 1 - 2s^2
    with all 2x/4x factors folded into a 4r copy and affine biases.
  - the spherical->xyz products are written by run-split paired muls
    DIRECTLY into the scan work buffer W's output-channel slots
    (no assembly copies); correction slots are reduced from W itself.
  - ONE gated tensor_tensor_scan per tile (3 components fused) computes
    the entire 22-edge kinematic tree walk in natural output-channel
    order:  state = gate*state + W;  gate=0 at root channels {0,1,6,11}
    reseeds state from the last observed frame, zero-offset slots
    reproduce the IGNORE copies, and two correction slots (ch16, ch24)
    rewind the state to x[13] across tree branches.
Output leaves the device component-major [nb, 3, 64, 32]; the host
transposes back to (B, T, 96).

Measured on trn2 (8 cores via axon): HW exec ~155 us/core,
relative error vs the jax reference: 1.1e-7.
"""

import math
import sys

for _p in ("/opt/trn_rl_repo", "/root/.axon_site/_ro/trn_rl_repo"):
    if _p not in sys.path:
        sys.path.insert(0, _p)

import numpy as np

PI = math.pi
BIGC = 1.5 * 2**23  # fp32 round-to-nearest-integer constant
T = 64   # time steps = reps per partition
P = 128  # partitions per tile

N_CORES = 8
B = 4096
NB = B // N_CORES  # batches per core
KERNEL_R = 32      # rows (time steps) per partition

# child-joint order of CONNECT (k index) -> contiguous runs in output-channel
# space: (k_start, ch_start, length)
ASSEM_RUNS = [
    (0, 12, 4),   # k0..3   -> ch12..15  (spine 12,13,14,15)
    (4, 25, 3),   # k4..6   -> ch25..27  (arm 25,26,27)
    (7, 29, 2),   # k7..8   -> ch29..30  (arm 29,30)
    (9, 17, 3),   # k9..11  -> ch17..19  (arm 17,18,19)
    (12, 21, 2),  # k12..13 -> ch21..22  (arm 21,22)
    (14, 2, 4),   # k14..17 -> ch2..5    (leg 2,3,4,5)
    (18, 7, 4),   # k18..21 -> ch7..10   (leg 7,8,9,10)
]


def build_kernel(nc, n_b: int, r: int = 32):
    """Build the postprocess kernel for n_b batch examples on one core."""
    import concourse.tile as tile
    from concourse import mybir

    f32 = mybir.dt.float32
    pred = nc.dram_tensor("pred", [n_b * T, 66], f32, kind="ExternalInput")
    # obs rows pre-duplicated on the host: row j <-> partition j of a tile
    # (each batch example spans 64//r partitions)
    obs = nc.dram_tensor("obs", [n_b * (T // r), 96], f32, kind="ExternalInput")
    out = nc.dram_tensor("out", [n_b, 3 * T * 32], f32, kind="ExternalOutput")

    with tile.TileContext(nc) as tc:
        build_tile_kernel(tc, pred, obs, out, n_b, r)
    return nc


def build_tile_kernel(tc, pred, obs, out, n_b: int, r: int):
    import concourse.bass as bass
    from concourse import mybir

    f32 = mybir.dt.float32
    ALU = mybir.AluOpType
    ACTF = mybir.ActivationFunctionType
    nc = tc.nc
    pp = min(P, n_b * T // r)
    nt = n_b * T // (pp * r)
    hb = r * 32          # out elems per partition per component
    KT = r * 22          # trig elems per half
    bpt = pp * r // T    # batch examples per tile

    # pred rows grouped per tile: partition p holds r consecutive rows
    pred_t = pred.ap().rearrange("(n p q) c -> n p (q c)", p=pp, q=r)

    with (
        tc.tile_pool(name="io", bufs=3) as io_pool,
        tc.tile_pool(name="mid", bufs=2) as mid_pool,
        tc.tile_pool(name="mid1", bufs=2) as mid1_pool,
        tc.tile_pool(name="const", bufs=1) as const_pool,
    ):
        # static gate tile [pp, 3*r*32] (all three components in one scan):
        # 1.0 everywhere, 0.0 at root channels
        gate = const_pool.tile([pp, 3 * hb], f32)
        g4 = gate.rearrange("p (c q j) -> p c q j", c=3, j=32)
        nc.vector.memset(gate, 1.0)
        nc.vector.memset(g4[:, :, :, 0:2], 0.0)       # ch 0, 1
        nc.vector.memset(g4[:, :, :, 6:12:5], 0.0)    # ch 6, 11

        # per-partition constant for Sin biases
        halfpi = const_pool.tile([pp, 1], f32)
        nc.vector.memset(halfpi, PI / 2)

        for i in range(nt):
            raw = io_pool.tile([pp, r * 66], f32)
            nc.sync.dma_start(out=raw, in_=pred_t[i])
            r4 = raw.rearrange("p (q c) -> p q c", c=66)
            # (theta, phi) strided view iterated (pair, rep, joint)
            th_ph_pm = bass.AP(tensor=raw.tensor, offset=raw.offset + 1,
                               ap=[raw.ap[0], [1, 2], [66, r], [3, 22]])

            # ---- trig via half-angle, no range reduction needed ----
            # |x/2| <= 2.85 < pi.  s = Sin(x/2), c = cos(x/2) = Sin(pi/2-|x|/2)
            # sin x = 2sc,  cos x = 1 - 2s^2; scale factors fold downstream.
            # TGH blocks: [s_t | s_p | c_t | c_p] (dense halves)
            tgh = mid1_pool.tile([pp, 4, KT], f32)
            half_out0 = bass.AP(tensor=tgh.tensor, offset=tgh.offset,
                                ap=[tgh.ap[0], [KT, 2], [22, r], [1, 22]])
            half_out2 = bass.AP(tensor=tgh.tensor, offset=tgh.offset + 2 * KT,
                                ap=[tgh.ap[0], [KT, 2], [22, r], [1, 22]])
            nc.scalar.activation(out=half_out0, in_=th_ph_pm, func=ACTF.Sin,
                                 bias=0.0, scale=0.5)
            absx = mid1_pool.tile([pp, 2, r, 22], f32)
            nc.scalar.activation(out=absx[:, :, :, :], in_=th_ph_pm,
                                 func=ACTF.Abs)
            nc.scalar.activation(out=half_out2, in_=absx[:, :, :, :],
                                 func=ACTF.Sin, bias=halfpi[:, 0:1], scale=-0.5)

            # TG2 blocks: [ct~ | st~ | sp~ | cp~]:
            #   st~ = s_t c_t (sin t = 2 st~),  ct~ = 0.5 - s_t^2
            #   sp~ = s_p c_p,                  cp~ = 0.25 - 0.5 s_p^2
            tg2 = mid1_pool.tile([pp, 4, r, 22], f32)
            nc.vector.tensor_tensor(
                out=bass.AP(tensor=tg2.tensor, offset=tg2.offset + KT,
                            ap=[tg2.ap[0], [KT, 2], [22, r], [1, 22]]),
                in0=tgh[:, 0:2], in1=tgh[:, 2:4], op=ALU.mult)
            sqt = mid1_pool.tile([pp, 2, KT], f32)
            nc.scalar.activation(out=sqt, in_=tgh[:, 0:2], func=ACTF.Square)
            nc.scalar.activation(out=tg2[:, 0], in_=sqt[:, 0].rearrange(
                "p (q k) -> p q k", k=22), func=ACTF.Copy, bias=0.5, scale=-1.0)
            nc.scalar.activation(out=tg2[:, 3], in_=sqt[:, 1].rearrange(
                "p (q k) -> p q k", k=22), func=ACTF.Copy, bias=0.25, scale=-0.5)

            # rd4 = 4r dense (GPSIMD copy of pre-scaled? no: ACT scale)
            rd = mid1_pool.tile([pp, r, 22], f32)
            nc.scalar.activation(out=rd, in_=r4[:, :, 0:66:3],
                                 func=ACTF.Copy, bias=0.0, scale=4.0)
            # rsin4 = 4r * sp~ = 2 r sin(phi)   (GPSIMD)
            rs = mid1_pool.tile([pp, r, 22], f32)
            nc.gpsimd.tensor_tensor(out=rs, in0=rd, in1=tg2[:, 2], op=ALU.mult)

            # ---- W [pp, 3, r, 32]: muls write x0/x1/x2 straight into
            # their output-channel slots (no assembly copies) ----
            w = io_pool.tile([pp, 3, r, 32], f32)
            nc.gpsimd.memset(w[:, :, :, 20:29:8], 0.0)
            nc.gpsimd.memset(w[:, :, :, 23:32:8], 0.0)
            hb32 = r * 32
            for k0, ch0, ln in ASSEM_RUNS:
                # [x0, x2] = [rsin4, rsin4] * [ct~, st~]  -> comps 0, 2  (DVE)
                nc.vector.tensor_tensor(
                    out=bass.AP(tensor=w.tensor, offset=w.offset + ch0,
                                ap=[w.ap[0], [2 * hb32, 2], [32, r], [1, ln]]),
                    in0=bass.AP(tensor=rs.tensor, offset=rs.offset + k0,
                                ap=[rs.ap[0], [0, 2], [22, r], [1, ln]]),
                    in1=bass.AP(tensor=tg2.tensor, offset=tg2.offset + k0,
                                ap=[tg2.ap[0], [KT, 2], [22, r], [1, ln]]),
                    op=ALU.mult)
                # x1 = 4r * cp~ = r cos(phi)  -> comp 1  (GPSIMD)
                nc.gpsimd.tensor_tensor(
                    out=w[:, 1, :, ch0 : ch0 + ln],
                    in0=rd[:, :, k0 : k0 + ln],
                    in1=tg2[:, 3, :, k0 : k0 + ln],
                    op=ALU.mult)

            # root slots from obs (host-duplicated rows: one row per partition)
            obs_t = mid_pool.tile([pp, 96], f32)
            nc.sync.dma_start(out=obs_t, in_=obs[i * pp : (i + 1) * pp, :])
            nc.scalar.copy(
                out=w[:, :, :, 0:2],
                in_=bass.AP(tensor=obs_t.tensor, offset=obs_t.offset,
                            ap=[obs_t.ap[0], [1, 3], [0, r], [3, 2]]),
            )
            nc.scalar.copy(
                out=w[:, :, :, 6:12:5],
                in_=bass.AP(tensor=obs_t.tensor, offset=obs_t.offset + 18,
                            ap=[obs_t.ap[0], [1, 3], [0, r], [15, 2]]),
            )
            # corrections, reading W itself:
            #   ch16 = -(W14+W15) = -(k2+k3);  ch24 = -(W17..W22) (W20 = 0)
            nc.vector.tensor_reduce(
                out=w[:, :, :, 16:17], in_=w[:, :, :, 14:16],
                axis=mybir.AxisListType.X, op=ALU.add, negate=True)
            nc.vector.tensor_reduce(
                out=w[:, :, :, 24:25], in_=w[:, :, :, 17:23],
                axis=mybir.AxisListType.X, op=ALU.add, negate=True)

            # ---- gated scan, all 3 components fused, out of place ----
            ot = io_pool.tile([pp, 3, hb], f32)
            nc.vector.tensor_tensor_scan(
                out=ot.rearrange("p c f -> p (c f)"), data0=gate,
                data1=w.rearrange("p c q j -> p (c q j)"),
                initial=0.0, op0=ALU.mult, op1=ALU.add)

            # out DMA per component: DRAM [b, c*2048 + t*32 + ch] with
            # b = i*bpt + p // (T//r), t = (p % (T//r))*r + rep
            for c in range(3):
                nc.sync.dma_start(
                    out=bass.AP(
                        tensor=out,
                        offset=(i * bpt) * (3 * T * 32) + c * (T * 32),
                        ap=[[3 * T * 32, bpt], [hb, T // r], [1, hb]],
                    ),
                    in_=ot[:, c],
                )


_CACHE = {}


def _get_nc():
    if "nc" not in _CACHE:
        import concourse.bacc as bacc

        nc = bacc.Bacc("TRN2", target_bir_lowering=False)
        build_kernel(nc, NB, r=KERNEL_R)
        nc.compile()
        _CACHE["nc"] = nc
    return _CACHE["nc"]


def _run(in_maps, **kwargs):
    from concourse.bass_utils import run_bass_kernel_spmd

    nc = _get_nc()
    return run_bass_kernel_spmd(nc, in_maps, core_ids=list(range(N_CORES)), **kwargs)


def _make_in_maps(observed_pose, pred_pose):
    obs_last = np.ascontiguousarray(observed_pose[:, -1, :], dtype=np.float32)
    # one obs row per tile-partition: duplicate each row T//r times
    obs_dup = np.repeat(obs_last, T // KERNEL_R, axis=0)
    pred = np.ascontiguousarray(pred_pose, dtype=np.float32)
    dup = T // KERNEL_R
    in_maps = []
    for c in range(N_CORES):
        in_maps.append(
            {
                "pred": np.ascontiguousarray(
                    pred[c * NB : (c + 1) * NB].reshape(NB * T, 66)
                ),
                "obs": np.ascontiguousarray(obs_dup[c * NB * dup : (c + 1) * NB * dup]),
            }
        )
    return in_maps


def _assemble_out(results):
    outs = []
    for c in range(N_CORES):
        o = results[c]["out"].reshape(NB, 3, T, 32)
        outs.append(o.transpose(0, 2, 3, 1).reshape(NB, T, 96))
    return np.ascontiguousarray(np.concatenate(outs, axis=0), dtype=np.float32)


def kernel(observed_pose, pred_pose):
    res = _run(_make_in_maps(observed_pose, pred_pose))
    return _assemble_out(res.results)


def kernel_traced(observed_pose, pred_pose, trace_cores=None):
    """Like kernel() but returns (output, BassKernelResults) with an NTFF trace."""
    res = _run(
        _make_in_maps(observed_pose, pred_pose),
        trace=True,
        trace_cores=trace_cores or [0],
    )
    return _assemble_out(res.results), res


# revision 24
# speedup vs baseline: 3.7064x; 1.0181x over previous
"""Trainium2 Bass kernel for the Human3.6M pose postprocess
(spherical->xyz conversion + kinematic-tree accumulation).

Self-contained: hardcodes the problem shapes
  observed_pose (4096, 16, 96) f32, pred_pose (4096, 64, 66) f32
and shards the batch dim across 8 NeuronCores (pure data parallel,
512 examples per core).

Per-core design (bt-major layout, no transposes, no TensorE):
  - partition p <-> 32 consecutive time rows of one batch example;
    free dim holds (time reps) x channels.  All DMA is >=4KB contiguous
    per partition.
  - trig via half-angle identities, so no range reduction is needed
    (|x/2| <= 2.85 < pi fits the Sin LUT):
      s = Sin(x/2), c = cos(x/2) = Sin(pi/2 - |x|/2)
      sin x = 2sc,  cos x = 1 - 2s^2
    with all 2x/4x factors folded into a 4r copy and affine biases.
  - the spherical->xyz products are written by run-split paired muls
    DIRECTLY into the scan work buffer W's output-channel slots
    (no assembly copies); correction slots are reduced from W itself.
  - ONE gated tensor_tensor_scan per tile (3 components fused) computes
    the entire 22-edge kinematic tree walk in natural output-channel
    order:  state = gate*state + W;  gate=0 at root channels {0,1,6,11}
    reseeds state from the last observed frame, zero-offset slots
    reproduce the IGNORE copies, and two correction slots (ch16, ch24)
    rewind the state to x[13] across tree branches.
Output leaves the device component-major [nb, 3, 64, 32]; the host
transposes back to (B, T, 96).

Measured on trn2 (8 cores via axon): HW exec ~155 us/core,
relative error vs the jax reference: 1.1e-7.
"""

import math
import sys

for _p in ("/opt/trn_rl_repo", "/root/.axon_site/_ro/trn_rl_repo"):
    if _p not in sys.path:
        sys.path.insert(0, _p)

import numpy as np

PI = math.pi
BIGC = 1.5 * 2**23  # fp32 round-to-nearest-integer constant
T = 64   # time steps = reps per partition
P = 128  # partitions per tile

N_CORES = 8
B = 4096
NB = B // N_CORES  # batches per core
KERNEL_R = 32      # rows (time steps) per partition

# child-joint order of CONNECT (k index) -> contiguous runs in output-channel
# space: (k_start, ch_start, length)
ASSEM_RUNS = [
    (0, 12, 4),   # k0..3   -> ch12..15  (spine 12,13,14,15)
    (4, 25, 3),   # k4..6   -> ch25..27  (arm 25,26,27)
    (7, 29, 2),   # k7..8   -> ch29..30  (arm 29,30)
    (9, 17, 3),   # k9..11  -> ch17..19  (arm 17,18,19)
    (12, 21, 2),  # k12..13 -> ch21..22  (arm 21,22)
    (14, 2, 4),   # k14..17 -> ch2..5    (leg 2,3,4,5)
    (18, 7, 4),   # k18..21 -> ch7..10   (leg 7,8,9,10)
]


def build_kernel(nc, n_b: int, r: int = 32):
    """Build the postprocess kernel for n_b batch examples on one core."""
    import concourse.tile as tile
    from concourse import mybir

    f32 = mybir.dt.float32
    pred = nc.dram_tensor("pred", [n_b * T, 66], f32, kind="ExternalInput")
    # obs rows pre-duplicated on the host: row j <-> partition j of a tile
    # (each batch example spans 64//r partitions)
    obs = nc.dram_tensor("obs", [n_b * (T // r), 96], f32, kind="ExternalInput")
    out = nc.dram_tensor("out", [n_b, 3 * T * 32], f32, kind="ExternalOutput")

    with tile.TileContext(nc) as tc:
        build_tile_kernel(tc, pred, obs, out, n_b, r)
    return nc


def build_tile_kernel(tc, pred, obs, out, n_b: int, r: int):
    import concourse.bass as bass
    from concourse import mybir

    f32 = mybir.dt.float32
    ALU = mybir.AluOpType
    ACTF = mybir.ActivationFunctionType
    nc = tc.nc
    pp = min(P, n_b * T // r)
    nt = n_b * T // (pp * r)
    hb = r * 32          # out elems per partition per component
    KT = r * 22          # trig elems per half
    bpt = pp * r // T    # batch examples per tile

    # pred rows grouped per tile: partition p holds r consecutive rows
    pred_t = pred.ap().rearrange("(n p q) c -> n p (q c)", p=pp, q=r)

    with (
        tc.tile_pool(name="io", bufs=3) as io_pool,
        tc.tile_pool(name="mid", bufs=2) as mid_pool,
        tc.tile_pool(name="mid1", bufs=2) as mid1_pool,
        tc.tile_pool(name="const", bufs=1) as const_pool,
    ):
        # static gate tile [pp, 3*r*32] (all three components in one scan):
        # 1.0 everywhere, 0.0 at root channels
        gate = const_pool.tile([pp, 3 * hb], f32)
        g4 = gate.rearrange("p (c q j) -> p c q j", c=3, j=32)
        nc.vector.memset(gate, 1.0)
        nc.vector.memset(g4[:, :, :, 0:2], 0.0)       # ch 0, 1
        nc.vector.memset(g4[:, :, :, 6:12:5], 0.0)    # ch 6, 11

        # per-partition constant for Sin biases
        halfpi = const_pool.tile([pp, 1], f32)
        nc.vector.memset(halfpi, PI / 2)

        for i in range(nt):
            raw = io_pool.tile([pp, r * 66], f32)
            nc.sync.dma_start(out=raw, in_=pred_t[i])
            r4 = raw.rearrange("p (q c) -> p q c", c=66)
            # (theta, phi) strided view iterated (pair, rep, joint)
            th_ph_pm = bass.AP(tensor=raw.tensor, offset=raw.offset + 1,
                               ap=[raw.ap[0], [1, 2], [66, r], [3, 22]])

            # ---- trig via half-angle, no range reduction needed ----
            # |x/2| <= 2.85 < pi.  s = Sin(x/2), c = cos(x/2) = Sin(pi/2-|x|/2)
            # sin x = 2sc,  cos x = 1 - 2s^2; scale factors fold downstream.
            # TGH blocks: [s_t | s_p | c_t | c_p] (dense halves)
            tgh = mid1_pool.tile([pp, 4, KT], f32)
            half_out0 = bass.AP(tensor=tgh.tensor, offset=tgh.offset,
                                ap=[tgh.ap[0], [KT, 2], [22, r], [1, 22]])
            half_out2 = bass.AP(tensor=tgh.tensor, offset=tgh.offset + 2 * KT,
                                ap=[tgh.ap[0], [KT, 2], [22, r], [1, 22]])
            nc.scalar.activation(out=half_out0, in_=th_ph_pm, func=ACTF.Sin,
                                 bias=0.0, scale=0.5)
            absx = mid1_pool.tile([pp, 2, r, 22], f32)
            nc.scalar.activation(out=absx[:, :, :, :], in_=th_ph_pm,
                                 func=ACTF.Abs)
            nc.scalar.activation(out=half_out2, in_=absx[:, :, :, :],
                                 func=ACTF.Sin, bias=halfpi[:, 0:1], scale=-0.5)

            # TG2 blocks: [ct~ | st~ | sp~ | cp~]:
            #   st~ = s_t c_t (sin t = 2 st~),  ct~ = 0.5 - s_t^2
            #   sp~ = s_p c_p,                  cp~ = 0.25 - 0.5 s_p^2
            tg2 = mid1_pool.tile([pp, 4, r, 22], f32)
            nc.vector.tensor_tensor(
                out=bass.AP(tensor=tg2.tensor, offset=tg2.offset + KT,
                            ap=[tg2.ap[0], [KT, 2], [22, r], [1, 22]]),
                in0=tgh[:, 0:2], in1=tgh[:, 2:4], op=ALU.mult)
            sqt = mid1_pool.tile([pp, 2, KT], f32)
            nc.scalar.activation(out=sqt, in_=tgh[:, 0:2], func=ACTF.Square)
            nc.scalar.activation(out=tg2[:, 0], in_=sqt[:, 0].rearrange(
                "p (q k) -> p q k", k=22), func=ACTF.Copy, bias=0.5, scale=-1.0)
            nc.scalar.activation(out=tg2[:, 3], in_=sqt[:, 1].rearrange(
                "p (q k) -> p q k", k=22), func=ACTF.Copy, bias=0.25, scale=-0.5)

            # rd4 = 4r dense (GPSIMD copy of pre-scaled? no: ACT scale)
            rd = mid1_pool.tile([pp, r, 22], f32)
            nc.scalar.activation(out=rd, in_=r4[:, :, 0:66:3],
                                 func=ACTF.Copy, bias=0.0, scale=4.0)
            # rsin4 = 4r * sp~ = 2 r sin(phi)   (GPSIMD)
            rs = mid1_pool.tile([pp, r, 22], f32)
            nc.gpsimd.tensor_tensor(out=rs, in0=rd, in1=tg2[:, 2], op=ALU.mult)

            # ---- W [pp, 3, r, 32]: muls write x0/x1/x2 straight into
            # their output-channel slots (no assembly copies) ----
            w = io_pool.tile([pp, 3, r, 32], f32)
            nc.gpsimd.memset(w[:, :, :, 20:29:8], 0.0)
            nc.gpsimd.memset(w[:, :, :, 23:32:8], 0.0)
            hb32 = r * 32
            for k0, ch0, ln in ASSEM_RUNS:
                # [x0, x2] = [rsin4, rsin4] * [ct~, st~]  -> comps 0, 2
                # (the two len-2 runs go to GPSIMD to shave the DVE serial path)
                eng = nc.gpsimd if ln == 2 else nc.vector
                eng.tensor_tensor(
                    out=bass.AP(tensor=w.tensor, offset=w.offset + ch0,
                                ap=[w.ap[0], [2 * hb32, 2], [32, r], [1, ln]]),
                    in0=bass.AP(tensor=rs.tensor, offset=rs.offset + k0,
                                ap=[rs.ap[0], [0, 2], [22, r], [1, ln]]),
                    in1=bass.AP(tensor=tg2.tensor, offset=tg2.offset + k0,
                                ap=[tg2.ap[0], [KT, 2], [22, r], [1, ln]]),
                    op=ALU.mult)
                # x1 = 4r * cp~ = r cos(phi)  -> comp 1  (DVE: tiny runs are
                # ~6x cheaper there than on GPSIMD's software dispatch)
                nc.vector.tensor_tensor(
                    out=w[:, 1, :, ch0 : ch0 + ln],
                    in0=rd[:, :, k0 : k0 + ln],
                    in1=tg2[:, 3, :, k0 : k0 + ln],
                    op=ALU.mult)

            # root slots from obs (host-duplicated rows: one row per partition)
            obs_t = mid_pool.tile([pp, 96], f32)
            nc.sync.dma_start(out=obs_t, in_=obs[i * pp : (i + 1) * pp, :])
            nc.scalar.copy(
                out=w[:, :, :, 0:2],
                in_=bass.AP(tensor=obs_t.tensor, offset=obs_t.offset,
                            ap=[obs_t.ap[0], [1, 3], [0, r], [3, 2]]),
            )
            nc.scalar.copy(
                out=w[:, :, :, 6:12:5],
                in_=bass.AP(tensor=obs_t.tensor, offset=obs_t.offset + 18,
                            ap=[obs_t.ap[0], [1, 3], [0, r], [15, 2]]),
            )
            # corrections, reading W itself:
            #   ch16 = -(W14+W15) = -(k2+k3);  ch24 = -(W17..W22) (W20 = 0)
            nc.vector.tensor_reduce(
                out=w[:, :, :, 16:17], in_=w[:, :, :, 14:16],
                axis=mybir.AxisListType.X, op=ALU.add, negate=True)
            nc.vector.tensor_reduce(
                out=w[:, :, :, 24:25], in_=w[:, :, :, 17:23],
                axis=mybir.AxisListType.X, op=ALU.add, negate=True)

            # ---- gated scan, all 3 components fused, out of place ----
            ot = io_pool.tile([pp, 3, hb], f32)
            nc.vector.tensor_tensor_scan(
                out=ot.rearrange("p c f -> p (c f)"), data0=gate,
                data1=w.rearrange("p c q j -> p (c q j)"),
                initial=0.0, op0=ALU.mult, op1=ALU.add)

            # out DMA per component: DRAM [b, c*2048 + t*32 + ch] with
            # b = i*bpt + p // (T//r), t = (p % (T//r))*r + rep
            for c in range(3):
                nc.sync.dma_start(
                    out=bass.AP(
                        tensor=out,
                        offset=(i * bpt) * (3 * T * 32) + c * (T * 32),
                        ap=[[3 * T * 32, bpt], [hb, T // r], [1, hb]],
                    ),
                    in_=ot[:, c],
                )


_CACHE = {}


def _get_nc():
    if "nc" not in _CACHE:
        import concourse.bacc as bacc

        nc = bacc.Bacc("TRN2", target_bir_lowering=False)
        build_kernel(nc, NB, r=KERNEL_R)
        nc.compile()
        _CACHE["nc"] = nc
    return _CACHE["nc"]


def _run(in_maps, **kwargs):
    from concourse.bass_utils import run_bass_kernel_spmd

    nc = _get_nc()
    return run_bass_kernel_spmd(nc, in_maps, core_ids=list(range(N_CORES)), **kwargs)


def _make_in_maps(observed_pose, pred_pose):
    obs_last = np.ascontiguousarray(observed_pose[:, -1, :], dtype=np.float32)
    # one obs row per tile-partition: duplicate each row T//r times
    obs_dup = np.repeat(obs_last, T // KERNEL_R, axis=0)
    pred = np.ascontiguousarray(pred_pose, dtype=np.float32)
    dup = T // KERNEL_R
    in_maps = []
    for c in range(N_CORES):
        in_maps.append(
            {
                "pred": np.ascontiguousarray(
                    pred[c * NB : (c + 1) * NB].reshape(NB * T, 66)
                ),
                "obs": np.ascontiguousarray(obs_dup[c * NB * dup : (c + 1) * NB * dup]),
            }
        )
    return in_maps


def _assemble_out(results):
    outs = []
    for c in range(N_CORES):
        o = results[c]["out"].reshape(NB, 3, T, 32)
        outs.append(o.transpose(0, 2, 3, 1).reshape(NB, T, 96))
    return np.ascontiguousarray(np.concatenate(outs, axis=0), dtype=np.float32)


def kernel(observed_pose, pred_pose):
    res = _run(_make_in_maps(observed_pose, pred_pose))
    return _assemble_out(res.results)


def kernel_traced(observed_pose, pred_pose, trace_cores=None):
    """Like kernel() but returns (output, BassKernelResults) with an NTFF trace."""
    res = _run(
        _make_in_maps(observed_pose, pred_pose),
        trace=True,
        trace_cores=trace_cores or [0],
    )
    return _assemble_out(res.results), res
